# revision 9
# baseline (speedup 1.0000x reference)
"""Trainium2 Bass kernel for bidirectional ActionLSTM.

Full inputs in, full output out. Internally: data-parallel over batch
(8 NeuronCores x 256 batch rows), LSTM weights replicated.

Device program (per core, transposed layout: hidden on partitions,
batch on the free dim):
  - fc_in is folded into the LSTM input weights on the host:
        W_x = w_ih @ fc_in_w  [512, 68],  bias = w_ih@fc_in_b + b_ih + b_hh
    and the bias is folded in as an extra all-ones input row (K=69).
  - Gate order permuted to [i, f, o, g]. tanh(g) is computed via the
    identity tanh(z) = 2*sigmoid(2z) - 1, with the 2z folded into the
    g-gate weight rows, so ONE sigmoid activation op covers all gates.
  - tanh(c) on ACT as a separate 256-col instruction (provably minimal:
    tanh(c(t)) must land strictly between SIG(t) and SIG(t+1)).
  - Mean-pool over time + fc_out done ON DEVICE: per step, two tiny
    PE matmuls project h through fc_out^T (M=3) and accumulate in one
    PSUM bank across all 128 steps.  This frees GpSimd completely
    (v2 ran hsum there, which stalled DVE h-writes ~1us/step via the
    shared SBUF port + lazy WAR semaphores).

v3 restructure vs v2:
  - PSUM: one shared 3-tile ring for both dirs' gates (6 banks) + 1
    bank for the fc_out projection accumulator.  Ring slot (d,t) is
    recycled 1.5 steps after its SIG consumes it.
  - GpSimd completely idle (no hsum); SBUF pools deepened (bufs=6) so
    WAR waits never land on the critical chain.
  - Emission in true temporal phase order per step:
      ACT:  SIG_f(t), TANH_b(t-1), TANH_f(t), SIG_b(t)
      DVE:  tt_f, m1h_f, h_b(t-1), add_f, h_f, tt_b, m1h_b, add_b
      PE :  U_b(t), proj_b(t-1), x_b(t+1), U_f(t+1), proj_f(t), x_f(t+2)
    so no in-order engine FIFO head-blocks the other direction's chain.
"""

import os
import numpy as np
from contextlib import ExitStack

INPUT, HID, NCLS = 68, 128, 3
B, T = 2048, 128
NCORES = 8
BL = B // NCORES          # 256 batch rows per core
KX = INPUT + 1            # 69 (ones row folds bias in)
KXP = 128                 # x-side K padded to 128: every matmul
                          # has the same [128,128] weight shape, so
                          # the PE never pays a shape-switch refill
G4 = 4 * HID              # 512

CELL_F32 = os.environ.get("LSTM_CELL_F32", "0") == "1"

_CACHE = {}


def _build_program():
    import concourse.bass as bass
    import concourse.tile as tile
    from concourse import bacc, mybir

    f32 = mybir.dt.float32
    bf16 = mybir.dt.bfloat16
    AF = mybir.ActivationFunctionType
    OP = mybir.AluOpType

    cell_dt = f32 if CELL_F32 else bf16

    nc = bacc.Bacc("TRN2", target_bir_lowering=False, debug=False,
                   num_devices=NCORES)

    xin = nc.dram_tensor("xin", [KXP, T * BL], bf16,
                         kind="ExternalInput").ap()
    wx = {}
    wu = {}
    wo = {}
    for d in "fb":
        wx[d] = nc.dram_tensor(f"wx_{d}", [KXP, G4], bf16,
                               kind="ExternalInput").ap()
        wu[d] = nc.dram_tensor(f"wu_{d}", [HID, G4], bf16,
                               kind="ExternalInput").ap()
        wo[d] = nc.dram_tensor(f"wo_{d}", [HID, NCLS], bf16,
                               kind="ExternalInput").ap()
    out = nc.dram_tensor("out", [NCLS, 2, 2, BL], f32,
                         kind="ExternalOutput").ap()

    with tile.TileContext(nc) as tc, ExitStack() as ctx:
        const = ctx.enter_context(tc.tile_pool(name="const", bufs=1))
        X = const.tile([KXP, T * BL], bf16, tag="X")

        # weights FIRST: they gate the very first matmuls, and anything
        # queued behind the big X transfer waits ~60us.
        WX = {}
        WU = {}
        WO = {}
        for d in "fb":
            WX[d] = const.tile([KXP, G4], bf16, tag=f"wx{d}", name=f"WX{d}")
            nc.sync.dma_start(WX[d][:], wx[d][:])
            WU[d] = const.tile([HID, G4], bf16, tag=f"wu{d}", name=f"WU{d}")
            nc.sync.dma_start(WU[d][:], wu[d][:])
            WO[d] = const.tile([HID, NCLS], bf16, tag=f"wo{d}", name=f"WO{d}")
            nc.sync.dma_start(WO[d][:], wo[d][:])

        # split the big input DMA into chunks so it spreads across DMA
        # queues and so early timesteps unblock compute quickly; issue
        # from both ends since the bwd direction consumes t=T-1 first.
        NCHUNK = 64
        CW = T * BL // NCHUNK
        order = []
        for i in range(NCHUNK // 2):
            order += [NCHUNK - 1 - i, i]
        for ci in order:
            nc.sync.dma_start(X[:, ci * CW:(ci + 1) * CW],
                              xin[:, ci * CW:(ci + 1) * CW])

        hpool = ctx.enter_context(tc.tile_pool(name="h", bufs=6))
        cpool = ctx.enter_context(tc.tile_pool(name="c", bufs=6))
        spool = ctx.enter_context(tc.tile_pool(name="s", bufs=6))
        scpool = ctx.enter_context(tc.tile_pool(name="sc", bufs=6))
        mpool = ctx.enter_context(tc.tile_pool(name="m1h", bufs=6))
        tpool = ctx.enter_context(tc.tile_pool(name="tt", bufs=6))
        gpsum = ctx.enter_context(tc.tile_pool(name="gates", bufs=3,
                                               space="PSUM"))
        ppsum = ctx.enter_context(tc.tile_pool(name="proj", bufs=1,
                                               space="PSUM"))

        proj = ppsum.tile([NCLS, 2, 2, BL], f32, tag="proj", name="proj")
        DI = {"f": 0, "b": 1}

        h = {}
        c = {}
        for d in "fb":
            c[d] = cpool.tile([HID, BL], cell_dt, tag=f"c{d}", name=f"c0{d}")
            nc.vector.memset(c[d][:], 0.0)

        g_cur = {}

        def emit_x(d, t):
            # x-side matmuls for step t into a fresh ring PSUM tile.
            # Each gate pair shares a 2KB PSUM bank: the even gate's
            # start=True matmul pending-zeroes the whole bank, the odd
            # gate accumulates into its (still pending-zero) half.
            g = gpsum.tile([HID, 4, BL], f32, tag="g", name=f"g_{d}_{t}")
            ts = t if d == "f" else T - 1 - t
            for gi in range(4):
                nc.tensor.matmul(g[:, gi, :],
                                 WX[d][:, gi * HID:(gi + 1) * HID],
                                 X[:, ts * BL:(ts + 1) * BL],
                                 start=(gi % 2 == 0),
                                 stop=(t == 0 and gi % 2 == 1),
                                 skip_group_check=True)
            return g

        def emit_U(d, t):
            g = g_cur[d]
            for gi in range(4):
                nc.tensor.matmul(g[:, gi, :],
                                 WU[d][:, gi * HID:(gi + 1) * HID],
                                 h[d], start=False,
                                 stop=(gi % 2 == 1),
                                 skip_group_check=True)

        def emit_sig(d, t, g):
            s = spool.tile([HID, 4, BL], cell_dt, tag=f"s{d}",
                           name=f"s{d}{t}")
            nc.scalar.activation(s[:], g[:], AF.Sigmoid)
            return s

        def emit_cell(d, t, s):
            # c_new = sig(f)*c + (sig(2g)-0.5)*sig(i)   [tt + m1h]
            tt = tpool.tile([HID, BL], cell_dt, tag=f"tt{d}",
                            name=f"tt{d}{t}")
            nc.vector.tensor_tensor(tt[:], s[:, 1, :], c[d][:], op=OP.mult)
            m1h = mpool.tile([HID, BL], cell_dt, tag=f"m{d}",
                             name=f"m{d}{t}")
            nc.vector.scalar_tensor_tensor(m1h[:], s[:, 3, :], 0.5,
                                           s[:, 0, :],
                                           op0=OP.subtract, op1=OP.mult)
            c_new = cpool.tile([HID, BL], cell_dt, tag=f"c{d}",
                               name=f"c{d}{t}")
            nc.vector.tensor_tensor(c_new[:], m1h[:], tt[:], op=OP.add)
            c[d] = c_new
            return c_new

        def emit_tanh(d, t, c_new):
            # sc = tanh(2*c_half) = tanh(c_true); shares the sigmoid
            # ACT table (no reload).
            sc = scpool.tile([HID, BL], cell_dt, tag=f"sc{d}",
                             name=f"sc{d}{t}")
            nc.scalar.activation(sc[:], c_new[:], AF.Tanh, scale=2.0)
            return sc

        ht = {}

        def emit_h(d, t, sc, s):
            if t % 2 == 0:
                ht[d] = hpool.tile([HID, 2, BL], bf16, tag=f"h{d}",
                                   name=f"h{d}{t}")
            nc.vector.tensor_tensor(ht[d][:, t % 2, :], sc[:], s[:, 2, :],
                                    op=OP.mult)
            h[d] = ht[d][:, t % 2, :]

        pending_proj = []

        def emit_proj(d, t):
            # defer emission ~2 iterations so the scheduler ranks these
            # after the chain-critical U matmuls of the next steps
            if t % 2 == 0:
                return
            pending_proj.append((d, t, ht[d]))

        def flush_proj(upto_t):
            while pending_proj and pending_proj[0][1] <= upto_t:
                d, t, htile = pending_proj.pop(0)
                nc.tensor.matmul(proj[:, DI[d], :, :], WO[d][:], htile[:],
                                 start=(t == 1), stop=(t == T - 1),
                                 skip_group_check=True)

        # ---- software-pipelined main loop ----
        # state carried between iterations (dir b runs half a phase
        # behind dir f in the steady schedule):
        g_cur["f"] = emit_x("f", 0)
        g_cur["b"] = emit_x("b", 0)

        s_b = None       # s tile of dir b from step t-1
        sc_b = None      # sc tile of dir b from step t-1
        for t in range(T):
            # --- dir f, step t ---
            if t > 0:
                emit_U("f", t)
            g_f = g_cur["f"]
            if t + 1 < T:
                g_cur["f"] = emit_x("f", t + 1)
            s_f = emit_sig("f", t, g_f)
            c_f = emit_cell("f", t, s_f)
            # dir b finishing step t-1: h_b, U_b of step t
            if t > 0:
                emit_h("b", t - 1, sc_b, s_b)
                emit_U("b", t)
                emit_proj("b", t - 1)
            sc_f = emit_tanh("f", t, c_f)
            # --- dir b, step t ---
            g_b = g_cur["b"]
            if t + 1 < T:
                g_cur["b"] = emit_x("b", t + 1)
            s_b = emit_sig("b", t, g_b)
            emit_h("f", t, sc_f, s_f)
            c_b = emit_cell("b", t, s_b)
            sc_b = emit_tanh("b", t, c_b)
            emit_proj("f", t)
            flush_proj(t - 6)
        # drain dir b's final step
        emit_h("b", T - 1, sc_b, s_b)
        emit_proj("b", T - 1)
        flush_proj(T)

        pout = const.tile([NCLS, 2, 2, BL], f32, tag="pout", name="pout")
        nc.vector.tensor_copy(pout[:], proj[:])
        nc.sync.dma_start(out[:], pout[:])

    nc.compile()
    return nc


def _prep_weights(w_ih, w_hh, b_ih, b_hh, fc_in_w, fc_in_b):
    Wx = w_ih.astype(np.float64) @ fc_in_w.astype(np.float64)   # [512, 68]
    bias = w_ih.astype(np.float64) @ fc_in_b.astype(np.float64) \
        + b_ih.astype(np.float64) + b_hh.astype(np.float64)
    perm = np.concatenate([np.arange(0, 128), np.arange(128, 256),
                           np.arange(384, 512), np.arange(256, 384)])
    Wx = Wx[perm]
    U = w_hh.astype(np.float64)[perm]
    bias = bias[perm]
    srow = np.ones((512, 1), np.float64)
    srow[384:] = 2.0
    Wx_aug = np.concatenate([Wx, bias[:, None]], axis=1)        # [512, 69]
    lhsT_x = np.ascontiguousarray((srow * Wx_aug).T)            # [69, 512]
    # h is stored as h = o*tanh(c_true): U scale = 1
    lhsT_U = np.ascontiguousarray((srow * U).T)                 # [128, 512]
    return lhsT_x, lhsT_U


def _pad_k(a):
    # [69, N] -> [128, N] zero-padded (the matching weight rows are zero)
    out = np.zeros((KXP, a.shape[1]), a.dtype)
    out[:a.shape[0]] = a
    return out


def kernel(x, fc_in_w, fc_in_b, w_ih_f, w_hh_f, b_ih_f, b_hh_f,
           w_ih_b, w_hh_b, b_ih_b, b_hh_b, fc_out_w, fc_out_b,
           _want_trace=False):
    from concourse import bass_utils
    import ml_dtypes

    bf16 = ml_dtypes.bfloat16

    if "nc" not in _CACHE:
        _CACHE["nc"] = _build_program()
    nc = _CACHE["nc"]

    lx_f, lU_f = _prep_weights(w_ih_f, w_hh_f, b_ih_f, b_hh_f,
                               fc_in_w, fc_in_b)
    lx_b, lU_b = _prep_weights(w_ih_b, w_hh_b, b_ih_b, b_hh_b,
                               fc_in_w, fc_in_b)
    # on-device pooling + fc_out: wo tiles are fc_out^T * (1/T)
    wo_f = np.ascontiguousarray(fc_out_w[:, :HID].astype(np.float64).T / T)
    wo_b = np.ascontiguousarray(fc_out_w[:, HID:].astype(np.float64).T / T)
    shared = {"wx_f": _pad_k(lx_f).astype(bf16),
              "wx_b": _pad_k(lx_b).astype(bf16),
              "wu_f": lU_f.astype(bf16),
              "wu_b": lU_b.astype(bf16),
              "wo_f": wo_f.astype(bf16),
              "wo_b": wo_b.astype(bf16)}

    in_maps = []
    for cidx in range(NCORES):
        xs = x[cidx * BL:(cidx + 1) * BL]                    # [BL, T, 68]
        xT = np.ascontiguousarray(xs.transpose(2, 1, 0))     # [68, T, BL]
        x_aug = np.concatenate(
            [xT, np.ones((1, T, BL), np.float32)], axis=0)   # [69, T, BL]
        x_aug = x_aug.reshape(KX, T * BL)
        xm = _pad_k(x_aug).astype(bf16)                      # [128, T*BL]
        in_maps.append({"xin": xm, **shared})

    res = bass_utils.run_bass_kernel_spmd(
        nc, in_maps, core_ids=list(range(NCORES)), trace=_want_trace)
    outs = []
    for cidx in range(NCORES):
        o = res.results[cidx]["out"].astype(np.float64)    # [3, 2, 2, BL]
        pool = o.sum(axis=(1, 2))                             # [3, BL]
        out_core = pool.T + fc_out_b                          # [BL, 3]
        outs.append(out_core)
    full = np.concatenate(outs, axis=0).astype(np.float32)
    if _want_trace:
        _CACHE["last_result"] = res
    return full


# revision 10
# speedup vs baseline: 1.0147x; 1.0147x over previous
"""Trainium2 Bass kernel for bidirectional ActionLSTM.

Full inputs in, full output out. Internally: data-parallel over batch
(8 NeuronCores x 256 batch rows), LSTM weights replicated.

Device program (per core, transposed layout: hidden on partitions,
batch on the free dim):
  - fc_in is folded into the LSTM input weights on the host:
        W_x = w_ih @ fc_in_w  [512, 68],  bias = w_ih@fc_in_b + b_ih + b_hh
    and the bias is folded in as an extra all-ones input row (K=69).
  - Gate order permuted to [i, f, o, g]. tanh(g) is computed via the
    identity tanh(z) = 2*sigmoid(2z) - 1, with the 2z folded into the
    g-gate weight rows, so ONE sigmoid activation op covers all gates.
  - tanh(c) on ACT as a separate 256-col instruction (provably minimal:
    tanh(c(t)) must land strictly between SIG(t) and SIG(t+1)).
  - Mean-pool over time + fc_out done ON DEVICE: per step, two tiny
    PE matmuls project h through fc_out^T (M=3) and accumulate in one
    PSUM bank across all 128 steps.  This frees GpSimd completely
    (v2 ran hsum there, which stalled DVE h-writes ~1us/step via the
    shared SBUF port + lazy WAR semaphores).

v3 restructure vs v2:
  - PSUM: one shared 3-tile ring for both dirs' gates (6 banks) + 1
    bank for the fc_out projection accumulator.  Ring slot (d,t) is
    recycled 1.5 steps after its SIG consumes it.
  - GpSimd completely idle (no hsum); SBUF pools deepened (bufs=6) so
    WAR waits never land on the critical chain.
  - Emission in true temporal phase order per step:
      ACT:  SIG_f(t), TANH_b(t-1), TANH_f(t), SIG_b(t)
      DVE:  tt_f, m1h_f, h_b(t-1), add_f, h_f, tt_b, m1h_b, add_b
      PE :  U_b(t), proj_b(t-1), x_b(t+1), U_f(t+1), proj_f(t), x_f(t+2)
    so no in-order engine FIFO head-blocks the other direction's chain.
"""

import os
import numpy as np
from contextlib import ExitStack

INPUT, HID, NCLS = 68, 128, 3
B, T = 2048, 128
NCORES = 8
BL = B // NCORES          # 256 batch rows per core
KX = INPUT + 1            # 69 (ones row folds bias in)
KXP = 128                 # x-side K padded to 128: every matmul
                          # has the same [128,128] weight shape, so
                          # the PE never pays a shape-switch refill
G4 = 4 * HID              # 512

CELL_F32 = os.environ.get("LSTM_CELL_F32", "0") == "1"

_CACHE = {}


def _build_program():
    import concourse.bass as bass
    import concourse.tile as tile
    from concourse import bacc, mybir

    f32 = mybir.dt.float32
    bf16 = mybir.dt.bfloat16
    AF = mybir.ActivationFunctionType
    OP = mybir.AluOpType

    cell_dt = f32 if CELL_F32 else bf16

    nc = bacc.Bacc("TRN2", target_bir_lowering=False, debug=False,
                   num_devices=NCORES)

    xin = nc.dram_tensor("xin", [KXP, T * BL], bf16,
                         kind="ExternalInput").ap()
    wx = {}
    wu = {}
    wo = {}
    for d in "fb":
        wx[d] = nc.dram_tensor(f"wx_{d}", [KXP, G4], bf16,
                               kind="ExternalInput").ap()
        wu[d] = nc.dram_tensor(f"wu_{d}", [HID, G4], bf16,
                               kind="ExternalInput").ap()
        wo[d] = nc.dram_tensor(f"wo_{d}", [HID, NCLS], bf16,
                               kind="ExternalInput").ap()
    out = nc.dram_tensor("out", [NCLS, 2, 2, BL], f32,
                         kind="ExternalOutput").ap()

    with tile.TileContext(nc) as tc, ExitStack() as ctx:
        const = ctx.enter_context(tc.tile_pool(name="const", bufs=1))
        X = const.tile([KXP, T * BL], bf16, tag="X")

        # weights FIRST: they gate the very first matmuls, and anything
        # queued behind the big X transfer waits ~60us.
        WX = {}
        WU = {}
        WO = {}
        for d in "fb":
            WX[d] = const.tile([KXP, G4], bf16, tag=f"wx{d}", name=f"WX{d}")
            nc.sync.dma_start(WX[d][:], wx[d][:])
            WU[d] = const.tile([HID, G4], bf16, tag=f"wu{d}", name=f"WU{d}")
            nc.sync.dma_start(WU[d][:], wu[d][:])
            WO[d] = const.tile([HID, NCLS], bf16, tag=f"wo{d}", name=f"WO{d}")
            nc.sync.dma_start(WO[d][:], wo[d][:])

        # split the big input DMA into chunks so it spreads across DMA
        # queues and so early timesteps unblock compute quickly; issue
        # from both ends since the bwd direction consumes t=T-1 first.
        NCHUNK = 64
        CW = T * BL // NCHUNK
        order = []
        for i in range(NCHUNK // 2):
            order += [NCHUNK - 1 - i, i]
        for ci in order:
            nc.sync.dma_start(X[:, ci * CW:(ci + 1) * CW],
                              xin[:, ci * CW:(ci + 1) * CW])

        hpool = ctx.enter_context(tc.tile_pool(name="h", bufs=6))
        cpool = ctx.enter_context(tc.tile_pool(name="c", bufs=6))
        spool = ctx.enter_context(tc.tile_pool(name="s", bufs=4))
        scpool = ctx.enter_context(tc.tile_pool(name="sc", bufs=6))
        mpool = ctx.enter_context(tc.tile_pool(name="m1h", bufs=6))
        tpool = ctx.enter_context(tc.tile_pool(name="tt", bufs=6))
        gpsum = ctx.enter_context(tc.tile_pool(name="gates", bufs=3,
                                               space="PSUM"))
        ppsum = ctx.enter_context(tc.tile_pool(name="proj", bufs=1,
                                               space="PSUM"))

        proj = ppsum.tile([NCLS, 2, 2, BL], f32, tag="proj", name="proj")
        DI = {"f": 0, "b": 1}

        h = {}
        c = {}
        for d in "fb":
            c[d] = cpool.tile([HID, BL], cell_dt, tag=f"c{d}", name=f"c0{d}")
            nc.vector.memset(c[d][:], 0.0)

        g_cur = {}

        def emit_x(d, t):
            # x-side matmuls for step t into a fresh ring PSUM tile.
            # Each gate pair shares a 2KB PSUM bank: the even gate's
            # start=True matmul pending-zeroes the whole bank, the odd
            # gate accumulates into its (still pending-zero) half.
            g = gpsum.tile([HID, 4, BL], f32, tag="g", name=f"g_{d}_{t}")
            ts = t if d == "f" else T - 1 - t
            for gi in range(4):
                nc.tensor.matmul(g[:, gi, :],
                                 WX[d][:, gi * HID:(gi + 1) * HID],
                                 X[:, ts * BL:(ts + 1) * BL],
                                 start=(gi % 2 == 0),
                                 stop=(t == 0 and gi % 2 == 1),
                                 skip_group_check=True)
            return g

        def emit_U(d, t):
            g = g_cur[d]
            for gi in range(4):
                nc.tensor.matmul(g[:, gi, :],
                                 WU[d][:, gi * HID:(gi + 1) * HID],
                                 h[d], start=False,
                                 stop=(gi % 2 == 1),
                                 skip_group_check=True)

        def emit_sig(d, t, g):
            s = spool.tile([HID, 4, BL], cell_dt, tag=f"s{d}",
                           name=f"s{d}{t}")
            nc.scalar.activation(s[:], g[:], AF.Sigmoid)
            return s

        def emit_cell(d, t, s):
            # c_new = sig(f)*c + (sig(2g)-0.5)*sig(i)   [tt + m1h]
            tt = tpool.tile([HID, BL], cell_dt, tag=f"tt{d}",
                            name=f"tt{d}{t}")
            nc.vector.tensor_tensor(tt[:], s[:, 1, :], c[d][:], op=OP.mult)
            m1h = mpool.tile([HID, BL], cell_dt, tag=f"m{d}",
                             name=f"m{d}{t}")
            nc.vector.scalar_tensor_tensor(m1h[:], s[:, 3, :], 0.5,
                                           s[:, 0, :],
                                           op0=OP.subtract, op1=OP.mult)
            c_new = cpool.tile([HID, BL], cell_dt, tag=f"c{d}",
                               name=f"c{d}{t}")
            nc.vector.tensor_tensor(c_new[:], m1h[:], tt[:], op=OP.add)
            c[d] = c_new
            return c_new

        def emit_tanh(d, t, c_new):
            # sc = tanh(2*c_half) = tanh(c_true); shares the sigmoid
            # ACT table (no reload).
            sc = scpool.tile([HID, BL], cell_dt, tag=f"sc{d}",
                             name=f"sc{d}{t}")
            nc.scalar.activation(sc[:], c_new[:], AF.Tanh, scale=2.0)
            return sc

        ht = {}

        def emit_h(d, t, sc, s):
            if t % 2 == 0:
                ht[d] = hpool.tile([HID, 2, BL], bf16, tag=f"h{d}",
                                   name=f"h{d}{t}")
            nc.vector.tensor_tensor(ht[d][:, t % 2, :], sc[:], s[:, 2, :],
                                    op=OP.mult)
            h[d] = ht[d][:, t % 2, :]

        pending_proj = []

        def emit_proj(d, t):
            # defer emission ~2 iterations so the scheduler ranks these
            # after the chain-critical U matmuls of the next steps
            if t % 2 == 0:
                return
            pending_proj.append((d, t, ht[d]))

        def flush_proj(upto_t):
            while pending_proj and pending_proj[0][1] <= upto_t:
                d, t, htile = pending_proj.pop(0)
                nc.tensor.matmul(proj[:, DI[d], :, :], WO[d][:], htile[:],
                                 start=(t == 1), stop=(t == T - 1),
                                 skip_group_check=True)

        # ---- software-pipelined main loop ----
        # state carried between iterations (dir b runs half a phase
        # behind dir f in the steady schedule):
        g_cur["f"] = emit_x("f", 0)
        g_cur["b"] = emit_x("b", 0)

        s_b = None       # s tile of dir b from step t-1
        sc_b = None      # sc tile of dir b from step t-1
        for t in range(T):
            # --- dir f, step t ---
            if t > 0:
                emit_U("f", t)
            g_f = g_cur["f"]
            if t + 1 < T:
                g_cur["f"] = emit_x("f", t + 1)
            s_f = emit_sig("f", t, g_f)
            c_f = emit_cell("f", t, s_f)
            # dir b finishing step t-1: h_b, U_b of step t
            if t > 0:
                emit_h("b", t - 1, sc_b, s_b)
                emit_U("b", t)
                emit_proj("b", t - 1)
            sc_f = emit_tanh("f", t, c_f)
            # --- dir b, step t ---
            g_b = g_cur["b"]
            if t + 1 < T:
                g_cur["b"] = emit_x("b", t + 1)
            s_b = emit_sig("b", t, g_b)
            emit_h("f", t, sc_f, s_f)
            c_b = emit_cell("b", t, s_b)
            sc_b = emit_tanh("b", t, c_b)
            emit_proj("f", t)
            flush_proj(t - 4)
        # drain dir b's final step
        emit_h("b", T - 1, sc_b, s_b)
        emit_proj("b", T - 1)
        flush_proj(T)

        pout = const.tile([NCLS, 2, 2, BL], f32, tag="pout", name="pout")
        nc.vector.tensor_copy(pout[:], proj[:])
        nc.sync.dma_start(out[:], pout[:])

    nc.compile()
    return nc


def _prep_weights(w_ih, w_hh, b_ih, b_hh, fc_in_w, fc_in_b):
    Wx = w_ih.astype(np.float64) @ fc_in_w.astype(np.float64)   # [512, 68]
    bias = w_ih.astype(np.float64) @ fc_in_b.astype(np.float64) \
        + b_ih.astype(np.float64) + b_hh.astype(np.float64)
    perm = np.concatenate([np.arange(0, 128), np.arange(128, 256),
                           np.arange(384, 512), np.arange(256, 384)])
    Wx = Wx[perm]
    U = w_hh.astype(np.float64)[perm]
    bias = bias[perm]
    srow = np.ones((512, 1), np.float64)
    srow[384:] = 2.0
    Wx_aug = np.concatenate([Wx, bias[:, None]], axis=1)        # [512, 69]
    lhsT_x = np.ascontiguousarray((srow * Wx_aug).T)            # [69, 512]
    # h is stored as h = o*tanh(c_true): U scale = 1
    lhsT_U = np.ascontiguousarray((srow * U).T)                 # [128, 512]
    return lhsT_x, lhsT_U


def _pad_k(a):
    # [69, N] -> [128, N] zero-padded (the matching weight rows are zero)
    out = np.zeros((KXP, a.shape[1]), a.dtype)
    out[:a.shape[0]] = a
    return out


def kernel(x, fc_in_w, fc_in_b, w_ih_f, w_hh_f, b_ih_f, b_hh_f,
           w_ih_b, w_hh_b, b_ih_b, b_hh_b, fc_out_w, fc_out_b,
           _want_trace=False):
    from concourse import bass_utils
    import ml_dtypes

    bf16 = ml_dtypes.bfloat16

    if "nc" not in _CACHE:
        _CACHE["nc"] = _build_program()
    nc = _CACHE["nc"]

    lx_f, lU_f = _prep_weights(w_ih_f, w_hh_f, b_ih_f, b_hh_f,
                               fc_in_w, fc_in_b)
    lx_b, lU_b = _prep_weights(w_ih_b, w_hh_b, b_ih_b, b_hh_b,
                               fc_in_w, fc_in_b)
    # on-device pooling + fc_out: wo tiles are fc_out^T * (1/T)
    wo_f = np.ascontiguousarray(fc_out_w[:, :HID].astype(np.float64).T / T)
    wo_b = np.ascontiguousarray(fc_out_w[:, HID:].astype(np.float64).T / T)
    shared = {"wx_f": _pad_k(lx_f).astype(bf16),
              "wx_b": _pad_k(lx_b).astype(bf16),
              "wu_f": lU_f.astype(bf16),
              "wu_b": lU_b.astype(bf16),
              "wo_f": wo_f.astype(bf16),
              "wo_b": wo_b.astype(bf16)}

    in_maps = []
    for cidx in range(NCORES):
        xs = x[cidx * BL:(cidx + 1) * BL]                    # [BL, T, 68]
        xT = np.ascontiguousarray(xs.transpose(2, 1, 0))     # [68, T, BL]
        x_aug = np.concatenate(
            [xT, np.ones((1, T, BL), np.float32)], axis=0)   # [69, T, BL]
        x_aug = x_aug.reshape(KX, T * BL)
        xm = _pad_k(x_aug).astype(bf16)                      # [128, T*BL]
        in_maps.append({"xin": xm, **shared})

    res = bass_utils.run_bass_kernel_spmd(
        nc, in_maps, core_ids=list(range(NCORES)), trace=_want_trace)
    outs = []
    for cidx in range(NCORES):
        o = res.results[cidx]["out"].astype(np.float64)    # [3, 2, 2, BL]
        pool = o.sum(axis=(1, 2))                             # [3, BL]
        out_core = pool.T + fc_out_b                          # [BL, 3]
        outs.append(out_core)
    full = np.concatenate(outs, axis=0).astype(np.float32)
    if _want_trace:
        _CACHE["last_result"] = res
    return full


# revision 11
# speedup vs baseline: 1.0179x; 1.0031x over previous
"""Trainium2 Bass kernel for bidirectional ActionLSTM.

Full inputs in, full output out. Internally: data-parallel over batch
(8 NeuronCores x 256 batch rows), LSTM weights replicated.

Device program (per core, transposed layout: hidden on partitions,
batch on the free dim):
  - fc_in is folded into the LSTM input weights on the host:
        W_x = w_ih @ fc_in_w  [512, 68],  bias = w_ih@fc_in_b + b_ih + b_hh
    and the bias is folded in as an extra all-ones input row (K=69).
  - Gate order permuted to [i, f, o, g]. tanh(g) is computed via the
    identity tanh(z) = 2*sigmoid(2z) - 1, with the 2z folded into the
    g-gate weight rows, so ONE sigmoid activation op covers all gates.
  - tanh(c) on ACT as a separate 256-col instruction (provably minimal:
    tanh(c(t)) must land strictly between SIG(t) and SIG(t+1)).
  - Mean-pool over time + fc_out done ON DEVICE: per step, two tiny
    PE matmuls project h through fc_out^T (M=3) and accumulate in one
    PSUM bank across all 128 steps.  This frees GpSimd completely
    (v2 ran hsum there, which stalled DVE h-writes ~1us/step via the
    shared SBUF port + lazy WAR semaphores).

v3 restructure vs v2:
  - PSUM: one shared 3-tile ring for both dirs' gates (6 banks) + 1
    bank for the fc_out projection accumulator.  Ring slot (d,t) is
    recycled 1.5 steps after its SIG consumes it.
  - GpSimd completely idle (no hsum); SBUF pools deepened (bufs=6) so
    WAR waits never land on the critical chain.
  - Emission in true temporal phase order per step:
      ACT:  SIG_f(t), TANH_b(t-1), TANH_f(t), SIG_b(t)
      DVE:  tt_f, m1h_f, h_b(t-1), add_f, h_f, tt_b, m1h_b, add_b
      PE :  U_b(t), proj_b(t-1), x_b(t+1), U_f(t+1), proj_f(t), x_f(t+2)
    so no in-order engine FIFO head-blocks the other direction's chain.
"""

import os
import numpy as np
from contextlib import ExitStack

INPUT, HID, NCLS = 68, 128, 3
B, T = 2048, 128
NCORES = 8
BL = B // NCORES          # 256 batch rows per core
KX = INPUT + 1            # 69 (ones row folds bias in)
KXP = 128                 # x-side K padded to 128: every matmul
                          # has the same [128,128] weight shape, so
                          # the PE never pays a shape-switch refill
G4 = 4 * HID              # 512

CELL_F32 = os.environ.get("LSTM_CELL_F32", "0") == "1"

_CACHE = {}


def _build_program():
    import concourse.bass as bass
    import concourse.tile as tile
    from concourse import bacc, mybir

    f32 = mybir.dt.float32
    bf16 = mybir.dt.bfloat16
    AF = mybir.ActivationFunctionType
    OP = mybir.AluOpType

    cell_dt = f32 if CELL_F32 else bf16

    nc = bacc.Bacc("TRN2", target_bir_lowering=False, debug=False,
                   num_devices=NCORES)

    xin = nc.dram_tensor("xin", [KXP, T * BL], bf16,
                         kind="ExternalInput").ap()
    wx = {}
    wu = {}
    wo = {}
    for d in "fb":
        wx[d] = nc.dram_tensor(f"wx_{d}", [KXP, G4], bf16,
                               kind="ExternalInput").ap()
        wu[d] = nc.dram_tensor(f"wu_{d}", [HID, G4], bf16,
                               kind="ExternalInput").ap()
        wo[d] = nc.dram_tensor(f"wo_{d}", [HID, NCLS], bf16,
                               kind="ExternalInput").ap()
    out = nc.dram_tensor("out", [NCLS, 2, 2, BL], f32,
                         kind="ExternalOutput").ap()

    with tile.TileContext(nc) as tc, ExitStack() as ctx:
        const = ctx.enter_context(tc.tile_pool(name="const", bufs=1))
        X = const.tile([KXP, T * BL], bf16, tag="X")

        # weights FIRST: they gate the very first matmuls, and anything
        # queued behind the big X transfer waits ~60us.
        WX = {}
        WU = {}
        WO = {}
        for d in "fb":
            WX[d] = const.tile([KXP, G4], bf16, tag=f"wx{d}", name=f"WX{d}")
            nc.sync.dma_start(WX[d][:], wx[d][:])
            WU[d] = const.tile([HID, G4], bf16, tag=f"wu{d}", name=f"WU{d}")
            nc.sync.dma_start(WU[d][:], wu[d][:])

        # split the big input DMA into chunks so it spreads across DMA
        # queues and so early timesteps unblock compute quickly; issue
        # from both ends since the bwd direction consumes t=T-1 first.
        NCHUNK = 64
        CW = T * BL // NCHUNK
        order = []
        for i in range(NCHUNK // 2):
            order += [NCHUNK - 1 - i, i]
        for ci in order[:8]:
            nc.sync.dma_start(X[:, ci * CW:(ci + 1) * CW],
                              xin[:, ci * CW:(ci + 1) * CW])
        for d in "fb":
            WO[d] = const.tile([HID, NCLS], bf16, tag=f"wo{d}", name=f"WO{d}")
            nc.sync.dma_start(WO[d][:], wo[d][:])
        for ci in order[8:]:
            nc.sync.dma_start(X[:, ci * CW:(ci + 1) * CW],
                              xin[:, ci * CW:(ci + 1) * CW])

        hpool = ctx.enter_context(tc.tile_pool(name="h", bufs=6))
        cpool = ctx.enter_context(tc.tile_pool(name="c", bufs=6))
        spool = ctx.enter_context(tc.tile_pool(name="s", bufs=4))
        scpool = ctx.enter_context(tc.tile_pool(name="sc", bufs=6))
        mpool = ctx.enter_context(tc.tile_pool(name="m1h", bufs=6))
        tpool = ctx.enter_context(tc.tile_pool(name="tt", bufs=6))
        gpsum = ctx.enter_context(tc.tile_pool(name="gates", bufs=3,
                                               space="PSUM"))
        ppsum = ctx.enter_context(tc.tile_pool(name="proj", bufs=1,
                                               space="PSUM"))

        proj = ppsum.tile([NCLS, 2, 2, BL], f32, tag="proj", name="proj")
        DI = {"f": 0, "b": 1}

        h = {}
        c = {}
        warm = scpool.tile([HID, 4], cell_dt, tag="warm", name="warm")
        nc.vector.memset(warm[:], 0.0)
        nc.scalar.activation(warm[:], warm[:], AF.Sigmoid)
        for d in "fb":
            c[d] = cpool.tile([HID, BL], cell_dt, tag=f"c{d}", name=f"c0{d}")
            nc.vector.memset(c[d][:], 0.0)

        g_cur = {}

        def emit_x(d, t):
            # x-side matmuls for step t into a fresh ring PSUM tile.
            # Each gate pair shares a 2KB PSUM bank: the even gate's
            # start=True matmul pending-zeroes the whole bank, the odd
            # gate accumulates into its (still pending-zero) half.
            g = gpsum.tile([HID, 4, BL], f32, tag="g", name=f"g_{d}_{t}")
            ts = t if d == "f" else T - 1 - t
            for gi in range(4):
                nc.tensor.matmul(g[:, gi, :],
                                 WX[d][:, gi * HID:(gi + 1) * HID],
                                 X[:, ts * BL:(ts + 1) * BL],
                                 start=(gi % 2 == 0),
                                 stop=(t == 0 and gi % 2 == 1),
                                 skip_group_check=True)
            return g

        def emit_U(d, t):
            g = g_cur[d]
            for gi in range(4):
                nc.tensor.matmul(g[:, gi, :],
                                 WU[d][:, gi * HID:(gi + 1) * HID],
                                 h[d], start=False,
                                 stop=(gi % 2 == 1),
                                 skip_group_check=True)

        def emit_sig(d, t, g):
            s = spool.tile([HID, 4, BL], cell_dt, tag=f"s{d}",
                           name=f"s{d}{t}")
            nc.scalar.activation(s[:], g[:], AF.Sigmoid)
            return s

        def emit_cell(d, t, s):
            # c_new = sig(f)*c + (sig(2g)-0.5)*sig(i)   [tt + m1h]
            tt = tpool.tile([HID, BL], cell_dt, tag=f"tt{d}",
                            name=f"tt{d}{t}")
            nc.vector.tensor_tensor(tt[:], s[:, 1, :], c[d][:], op=OP.mult)
            m1h = mpool.tile([HID, BL], cell_dt, tag=f"m{d}",
                             name=f"m{d}{t}")
            nc.vector.scalar_tensor_tensor(m1h[:], s[:, 3, :], 0.5,
                                           s[:, 0, :],
                                           op0=OP.subtract, op1=OP.mult)
            c_new = cpool.tile([HID, BL], cell_dt, tag=f"c{d}",
                               name=f"c{d}{t}")
            nc.vector.tensor_tensor(c_new[:], m1h[:], tt[:], op=OP.add)
            c[d] = c_new
            return c_new

        def emit_tanh(d, t, c_new):
            # sc = tanh(2*c_half) = tanh(c_true); shares the sigmoid
            # ACT table (no reload).
            sc = scpool.tile([HID, BL], cell_dt, tag=f"sc{d}",
                             name=f"sc{d}{t}")
            nc.scalar.activation(sc[:], c_new[:], AF.Tanh, scale=2.0)
            return sc

        ht = {}

        def emit_h(d, t, sc, s):
            if t % 2 == 0:
                ht[d] = hpool.tile([HID, 2, BL], bf16, tag=f"h{d}",
                                   name=f"h{d}{t}")
            nc.vector.tensor_tensor(ht[d][:, t % 2, :], sc[:], s[:, 2, :],
                                    op=OP.mult)
            h[d] = ht[d][:, t % 2, :]

        pending_proj = []

        def emit_proj(d, t):
            # defer emission ~2 iterations so the scheduler ranks these
            # after the chain-critical U matmuls of the next steps
            if t % 2 == 0:
                return
            pending_proj.append((d, t, ht[d]))

        def flush_proj(upto_t):
            while pending_proj and pending_proj[0][1] <= upto_t:
                d, t, htile = pending_proj.pop(0)
                nc.tensor.matmul(proj[:, DI[d], :, :], WO[d][:], htile[:],
                                 start=(t == 1), stop=(t == T - 1),
                                 skip_group_check=True)

        # ---- software-pipelined main loop ----
        # state carried between iterations (dir b runs half a phase
        # behind dir f in the steady schedule):
        g_cur["f"] = emit_x("f", 0)
        g_cur["b"] = emit_x("b", 0)

        s_b = None       # s tile of dir b from step t-1
        sc_b = None      # sc tile of dir b from step t-1
        for t in range(T):
            # --- dir f, step t ---
            if t > 0:
                emit_U("f", t)
            g_f = g_cur["f"]
            if t + 1 < T:
                g_cur["f"] = emit_x("f", t + 1)
            s_f = emit_sig("f", t, g_f)
            c_f = emit_cell("f", t, s_f)
            # dir b finishing step t-1: h_b, U_b of step t
            if t > 0:
                emit_h("b", t - 1, sc_b, s_b)
                emit_U("b", t)
                emit_proj("b", t - 1)
            sc_f = emit_tanh("f", t, c_f)
            # --- dir b, step t ---
            g_b = g_cur["b"]
            if t + 1 < T:
                g_cur["b"] = emit_x("b", t + 1)
            s_b = emit_sig("b", t, g_b)
            emit_h("f", t, sc_f, s_f)
            c_b = emit_cell("b", t, s_b)
            sc_b = emit_tanh("b", t, c_b)
            emit_proj("f", t)
            flush_proj(t - 4 if t < T - 3 else t - 1)
        # drain dir b's final step
        emit_h("b", T - 1, sc_b, s_b)
        emit_proj("b", T - 1)
        flush_proj(T)

        pout = const.tile([NCLS, 2, 2, BL], f32, tag="pout", name="pout")
        nc.vector.tensor_copy(pout[:], proj[:])
        nc.sync.dma_start(out[:], pout[:])

    nc.compile()
    return nc


def _prep_weights(w_ih, w_hh, b_ih, b_hh, fc_in_w, fc_in_b):
    Wx = w_ih.astype(np.float64) @ fc_in_w.astype(np.float64)   # [512, 68]
    bias = w_ih.astype(np.float64) @ fc_in_b.astype(np.float64) \
        + b_ih.astype(np.float64) + b_hh.astype(np.float64)
    perm = np.concatenate([np.arange(0, 128), np.arange(128, 256),
                           np.arange(384, 512), np.arange(256, 384)])
    Wx = Wx[perm]
    U = w_hh.astype(np.float64)[perm]
    bias = bias[perm]
    srow = np.ones((512, 1), np.float64)
    srow[384:] = 2.0
    Wx_aug = np.concatenate([Wx, bias[:, None]], axis=1)        # [512, 69]
    lhsT_x = np.ascontiguousarray((srow * Wx_aug).T)            # [69, 512]
    # h is stored as h = o*tanh(c_true): U scale = 1
    lhsT_U = np.ascontiguousarray((srow * U).T)                 # [128, 512]
    return lhsT_x, lhsT_U


def _pad_k(a):
    # [69, N] -> [128, N] zero-padded (the matching weight rows are zero)
    out = np.zeros((KXP, a.shape[1]), a.dtype)
    out[:a.shape[0]] = a
    return out


def kernel(x, fc_in_w, fc_in_b, w_ih_f, w_hh_f, b_ih_f, b_hh_f,
           w_ih_b, w_hh_b, b_ih_b, b_hh_b, fc_out_w, fc_out_b,
           _want_trace=False):
    from concourse import bass_utils
    import ml_dtypes

    bf16 = ml_dtypes.bfloat16

    if "nc" not in _CACHE:
        _CACHE["nc"] = _build_program()
    nc = _CACHE["nc"]

    lx_f, lU_f = _prep_weights(w_ih_f, w_hh_f, b_ih_f, b_hh_f,
                               fc_in_w, fc_in_b)
    lx_b, lU_b = _prep_weights(w_ih_b, w_hh_b, b_ih_b, b_hh_b,
                               fc_in_w, fc_in_b)
    # on-device pooling + fc_out: wo tiles are fc_out^T * (1/T)
    wo_f = np.ascontiguousarray(fc_out_w[:, :HID].astype(np.float64).T / T)
    wo_b = np.ascontiguousarray(fc_out_w[:, HID:].astype(np.float64).T / T)
    shared = {"wx_f": _pad_k(lx_f).astype(bf16),
              "wx_b": _pad_k(lx_b).astype(bf16),
              "wu_f": lU_f.astype(bf16),
              "wu_b": lU_b.astype(bf16),
              "wo_f": wo_f.astype(bf16),
              "wo_b": wo_b.astype(bf16)}

    in_maps = []
    for cidx in range(NCORES):
        xs = x[cidx * BL:(cidx + 1) * BL]                    # [BL, T, 68]
        xT = np.ascontiguousarray(xs.transpose(2, 1, 0))     # [68, T, BL]
        x_aug = np.concatenate(
            [xT, np.ones((1, T, BL), np.float32)], axis=0)   # [69, T, BL]
        x_aug = x_aug.reshape(KX, T * BL)
        xm = _pad_k(x_aug).astype(bf16)                      # [128, T*BL]
        in_maps.append({"xin": xm, **shared})

    res = bass_utils.run_bass_kernel_spmd(
        nc, in_maps, core_ids=list(range(NCORES)), trace=_want_trace)
    outs = []
    for cidx in range(NCORES):
        o = res.results[cidx]["out"].astype(np.float64)    # [3, 2, 2, BL]
        pool = o.sum(axis=(1, 2))                             # [3, BL]
        out_core = pool.T + fc_out_b                          # [BL, 3]
        outs.append(out_core)
    full = np.concatenate(outs, axis=0).astype(np.float32)
    if _want_trace:
        _CACHE["last_result"] = res
    return full


# revision 12
# speedup vs baseline: 1.0184x; 1.0005x over previous
"""Trainium2 Bass kernel for bidirectional ActionLSTM.

Full inputs in, full output out. Internally: data-parallel over batch
(8 NeuronCores x 256 batch rows), LSTM weights replicated.

Device program (per core, transposed layout: hidden on partitions,
batch on the free dim):
  - fc_in is folded into the LSTM input weights on the host:
        W_x = w_ih @ fc_in_w  [512, 68],  bias = w_ih@fc_in_b + b_ih + b_hh
    and the bias is folded in as an extra all-ones input row (K=69).
  - Gate order permuted to [i, f, o, g]. tanh(g) is computed via the
    identity tanh(z) = 2*sigmoid(2z) - 1, with the 2z folded into the
    g-gate weight rows, so ONE sigmoid activation op covers all gates.
  - tanh(c) on ACT as a separate 256-col instruction (provably minimal:
    tanh(c(t)) must land strictly between SIG(t) and SIG(t+1)).
  - Mean-pool over time + fc_out done ON DEVICE: per step, two tiny
    PE matmuls project h through fc_out^T (M=3) and accumulate in one
    PSUM bank across all 128 steps.  This frees GpSimd completely
    (v2 ran hsum there, which stalled DVE h-writes ~1us/step via the
    shared SBUF port + lazy WAR semaphores).

v3 restructure vs v2:
  - PSUM: one shared 3-tile ring for both dirs' gates (6 banks) + 1
    bank for the fc_out projection accumulator.  Ring slot (d,t) is
    recycled 1.5 steps after its SIG consumes it.
  - GpSimd completely idle (no hsum); SBUF pools deepened (bufs=6) so
    WAR waits never land on the critical chain.
  - Emission in true temporal phase order per step:
      ACT:  SIG_f(t), TANH_b(t-1), TANH_f(t), SIG_b(t)
      DVE:  tt_f, m1h_f, h_b(t-1), add_f, h_f, tt_b, m1h_b, add_b
      PE :  U_b(t), proj_b(t-1), x_b(t+1), U_f(t+1), proj_f(t), x_f(t+2)
    so no in-order engine FIFO head-blocks the other direction's chain.
"""

import os
import numpy as np
from contextlib import ExitStack

INPUT, HID, NCLS = 68, 128, 3
B, T = 2048, 128
NCORES = 8
BL = B // NCORES          # 256 batch rows per core
KX = INPUT + 1            # 69 (ones row folds bias in)
KXP = 128                 # x-side K padded to 128: every matmul
                          # has the same [128,128] weight shape, so
                          # the PE never pays a shape-switch refill
G4 = 4 * HID              # 512

CELL_F32 = os.environ.get("LSTM_CELL_F32", "0") == "1"

_CACHE = {}


def _build_program():
    import concourse.bass as bass
    import concourse.tile as tile
    from concourse import bacc, mybir

    f32 = mybir.dt.float32
    bf16 = mybir.dt.bfloat16
    AF = mybir.ActivationFunctionType
    OP = mybir.AluOpType

    cell_dt = f32 if CELL_F32 else bf16

    nc = bacc.Bacc("TRN2", target_bir_lowering=False, debug=False,
                   num_devices=NCORES)

    xin = nc.dram_tensor("xin", [KXP, T * BL], bf16,
                         kind="ExternalInput").ap()
    wx = {}
    wu = {}
    wo = {}
    for d in "fb":
        wx[d] = nc.dram_tensor(f"wx_{d}", [KXP, G4], bf16,
                               kind="ExternalInput").ap()
        wu[d] = nc.dram_tensor(f"wu_{d}", [HID, G4], bf16,
                               kind="ExternalInput").ap()
        wo[d] = nc.dram_tensor(f"wo_{d}", [HID, NCLS], bf16,
                               kind="ExternalInput").ap()
    out = nc.dram_tensor("out", [NCLS, 2, 2, BL], f32,
                         kind="ExternalOutput").ap()

    with tile.TileContext(nc) as tc, ExitStack() as ctx:
        const = ctx.enter_context(tc.tile_pool(name="const", bufs=1))
        X = const.tile([KXP, T * BL], bf16, tag="X")

        # weights FIRST: they gate the very first matmuls, and anything
        # queued behind the big X transfer waits ~60us.
        WX = {}
        WU = {}
        WO = {}
        for d in "fb":
            WX[d] = const.tile([KXP, G4], bf16, tag=f"wx{d}", name=f"WX{d}")
            nc.sync.dma_start(WX[d][:], wx[d][:])
            WU[d] = const.tile([HID, G4], bf16, tag=f"wu{d}", name=f"WU{d}")
            nc.sync.dma_start(WU[d][:], wu[d][:])

        # split the big input DMA into chunks so it spreads across DMA
        # queues and so early timesteps unblock compute quickly; issue
        # from both ends since the bwd direction consumes t=T-1 first.
        NCHUNK = 64
        CW = T * BL // NCHUNK
        order = []
        for i in range(NCHUNK // 2):
            order += [i, NCHUNK - 1 - i]
        for ci in order[:8]:
            nc.sync.dma_start(X[:, ci * CW:(ci + 1) * CW],
                              xin[:, ci * CW:(ci + 1) * CW])
        for d in "fb":
            WO[d] = const.tile([HID, NCLS], bf16, tag=f"wo{d}", name=f"WO{d}")
            nc.sync.dma_start(WO[d][:], wo[d][:])
        for ci in order[8:]:
            nc.sync.dma_start(X[:, ci * CW:(ci + 1) * CW],
                              xin[:, ci * CW:(ci + 1) * CW])

        hpool = ctx.enter_context(tc.tile_pool(name="h", bufs=6))
        cpool = ctx.enter_context(tc.tile_pool(name="c", bufs=6))
        spool = ctx.enter_context(tc.tile_pool(name="s", bufs=4))
        scpool = ctx.enter_context(tc.tile_pool(name="sc", bufs=6))
        mpool = ctx.enter_context(tc.tile_pool(name="m1h", bufs=6))
        tpool = ctx.enter_context(tc.tile_pool(name="tt", bufs=6))
        gpsum = ctx.enter_context(tc.tile_pool(name="gates", bufs=3,
                                               space="PSUM"))
        ppsum = ctx.enter_context(tc.tile_pool(name="proj", bufs=1,
                                               space="PSUM"))

        proj = ppsum.tile([NCLS, 2, 2, BL], f32, tag="proj", name="proj")
        DI = {"f": 0, "b": 1}

        h = {}
        c = {}
        warm = scpool.tile([HID, 4], cell_dt, tag="warm", name="warm")
        nc.vector.memset(warm[:], 0.0)
        nc.scalar.activation(warm[:], warm[:], AF.Sigmoid)
        jw = const.tile([KXP, BL], bf16, tag="jw", name="jw")
        nc.vector.memset(jw[:], 0.0)
        jg = gpsum.tile([HID, 4, BL], f32, tag="g", name="warmg")
        for i in range(16):
            nc.tensor.matmul(jg[:, i % 4, :], jw[:, 0:HID], jw[:],
                             start=True, stop=(i == 15),
                             skip_group_check=True)
        for d in "fb":
            c[d] = cpool.tile([HID, BL], cell_dt, tag=f"c{d}", name=f"c0{d}")
            nc.vector.memset(c[d][:], 0.0)

        g_cur = {}

        def emit_x(d, t):
            # x-side matmuls for step t into a fresh ring PSUM tile.
            # Each gate pair shares a 2KB PSUM bank: the even gate's
            # start=True matmul pending-zeroes the whole bank, the odd
            # gate accumulates into its (still pending-zero) half.
            g = gpsum.tile([HID, 4, BL], f32, tag="g", name=f"g_{d}_{t}")
            ts = t if d == "f" else T - 1 - t
            for gi in range(4):
                nc.tensor.matmul(g[:, gi, :],
                                 WX[d][:, gi * HID:(gi + 1) * HID],
                                 X[:, ts * BL:(ts + 1) * BL],
                                 start=(gi % 2 == 0),
                                 stop=(t == 0 and gi % 2 == 1),
                                 skip_group_check=True)
            return g

        def emit_U(d, t):
            g = g_cur[d]
            for gi in range(4):
                nc.tensor.matmul(g[:, gi, :],
                                 WU[d][:, gi * HID:(gi + 1) * HID],
                                 h[d], start=False,
                                 stop=(gi % 2 == 1),
                                 skip_group_check=True)

        def emit_sig(d, t, g):
            s = spool.tile([HID, 4, BL], cell_dt, tag=f"s{d}",
                           name=f"s{d}{t}")
            nc.scalar.activation(s[:], g[:], AF.Sigmoid)
            return s

        def emit_cell(d, t, s):
            # c_new = sig(f)*c + (sig(2g)-0.5)*sig(i)   [tt + m1h]
            tt = tpool.tile([HID, BL], cell_dt, tag=f"tt{d}",
                            name=f"tt{d}{t}")
            nc.vector.tensor_tensor(tt[:], s[:, 1, :], c[d][:], op=OP.mult)
            m1h = mpool.tile([HID, BL], cell_dt, tag=f"m{d}",
                             name=f"m{d}{t}")
            nc.vector.scalar_tensor_tensor(m1h[:], s[:, 3, :], 0.5,
                                           s[:, 0, :],
                                           op0=OP.subtract, op1=OP.mult)
            c_new = cpool.tile([HID, BL], cell_dt, tag=f"c{d}",
                               name=f"c{d}{t}")
            nc.vector.tensor_tensor(c_new[:], m1h[:], tt[:], op=OP.add)
            c[d] = c_new
            return c_new

        def emit_tanh(d, t, c_new):
            # sc = tanh(2*c_half) = tanh(c_true); shares the sigmoid
            # ACT table (no reload).
            sc = scpool.tile([HID, BL], cell_dt, tag=f"sc{d}",
                             name=f"sc{d}{t}")
            nc.scalar.activation(sc[:], c_new[:], AF.Tanh, scale=2.0)
            return sc

        ht = {}

        def emit_h(d, t, sc, s):
            if t % 2 == 0:
                ht[d] = hpool.tile([HID, 2, BL], bf16, tag=f"h{d}",
                                   name=f"h{d}{t}")
            nc.vector.tensor_tensor(ht[d][:, t % 2, :], sc[:], s[:, 2, :],
                                    op=OP.mult)
            h[d] = ht[d][:, t % 2, :]

        pending_proj = []

        def emit_proj(d, t):
            # defer emission ~2 iterations so the scheduler ranks these
            # after the chain-critical U matmuls of the next steps
            if t % 2 == 0:
                return
            pending_proj.append((d, t, ht[d]))

        def flush_proj(upto_t):
            while pending_proj and pending_proj[0][1] <= upto_t:
                d, t, htile = pending_proj.pop(0)
                nc.tensor.matmul(proj[:, DI[d], :, :], WO[d][:], htile[:],
                                 start=(t == 1), stop=(t == T - 1),
                                 skip_group_check=True)

        # ---- software-pipelined main loop ----
        # state carried between iterations (dir b runs half a phase
        # behind dir f in the steady schedule):
        g_cur["f"] = emit_x("f", 0)
        g_cur["b"] = emit_x("b", 0)

        s_b = None       # s tile of dir b from step t-1
        sc_b = None      # sc tile of dir b from step t-1
        for t in range(T):
            # --- dir f, step t ---
            if t > 0:
                emit_U("f", t)
            g_f = g_cur["f"]
            if t + 1 < T:
                g_cur["f"] = emit_x("f", t + 1)
            s_f = emit_sig("f", t, g_f)
            c_f = emit_cell("f", t, s_f)
            # dir b finishing step t-1: h_b, U_b of step t
            if t > 0:
                emit_h("b", t - 1, sc_b, s_b)
                emit_U("b", t)
                emit_proj("b", t - 1)
            sc_f = emit_tanh("f", t, c_f)
            # --- dir b, step t ---
            g_b = g_cur["b"]
            if t + 1 < T:
                g_cur["b"] = emit_x("b", t + 1)
            s_b = emit_sig("b", t, g_b)
            emit_h("f", t, sc_f, s_f)
            c_b = emit_cell("b", t, s_b)
            sc_b = emit_tanh("b", t, c_b)
            emit_proj("f", t)
            flush_proj(t - 4 if t < T - 3 else t - 1)
        # drain dir b's final step
        emit_h("b", T - 1, sc_b, s_b)
        emit_proj("b", T - 1)
        flush_proj(T)

        pout = const.tile([NCLS, 2, 2, BL], f32, tag="pout", name="pout")
        nc.vector.tensor_copy(pout[:], proj[:])
        nc.sync.dma_start(out[:], pout[:])

    nc.compile()
    return nc


def _prep_weights(w_ih, w_hh, b_ih, b_hh, fc_in_w, fc_in_b):
    Wx = w_ih.astype(np.float64) @ fc_in_w.astype(np.float64)   # [512, 68]
    bias = w_ih.astype(np.float64) @ fc_in_b.astype(np.float64) \
        + b_ih.astype(np.float64) + b_hh.astype(np.float64)
    perm = np.concatenate([np.arange(0, 128), np.arange(128, 256),
                           np.arange(384, 512), np.arange(256, 384)])
    Wx = Wx[perm]
    U = w_hh.astype(np.float64)[perm]
    bias = bias[perm]
    srow = np.ones((512, 1), np.float64)
    srow[384:] = 2.0
    Wx_aug = np.concatenate([Wx, bias[:, None]], axis=1)        # [512, 69]
    lhsT_x = np.ascontiguousarray((srow * Wx_aug).T)            # [69, 512]
    # h is stored as h = o*tanh(c_true): U scale = 1
    lhsT_U = np.ascontiguousarray((srow * U).T)                 # [128, 512]
    return lhsT_x, lhsT_U


def _pad_k(a):
    # [69, N] -> [128, N] zero-padded (the matching weight rows are zero)
    out = np.zeros((KXP, a.shape[1]), a.dtype)
    out[:a.shape[0]] = a
    return out


def kernel(x, fc_in_w, fc_in_b, w_ih_f, w_hh_f, b_ih_f, b_hh_f,
           w_ih_b, w_hh_b, b_ih_b, b_hh_b, fc_out_w, fc_out_b,
           _want_trace=False):
    from concourse import bass_utils
    import ml_dtypes

    bf16 = ml_dtypes.bfloat16

    if "nc" not in _CACHE:
        _CACHE["nc"] = _build_program()
    nc = _CACHE["nc"]

    lx_f, lU_f = _prep_weights(w_ih_f, w_hh_f, b_ih_f, b_hh_f,
                               fc_in_w, fc_in_b)
    lx_b, lU_b = _prep_weights(w_ih_b, w_hh_b, b_ih_b, b_hh_b,
                               fc_in_w, fc_in_b)
    # on-device pooling + fc_out: wo tiles are fc_out^T * (1/T)
    wo_f = np.ascontiguousarray(fc_out_w[:, :HID].astype(np.float64).T / T)
    wo_b = np.ascontiguousarray(fc_out_w[:, HID:].astype(np.float64).T / T)
    shared = {"wx_f": _pad_k(lx_f).astype(bf16),
              "wx_b": _pad_k(lx_b).astype(bf16),
              "wu_f": lU_f.astype(bf16),
              "wu_b": lU_b.astype(bf16),
              "wo_f": wo_f.astype(bf16),
              "wo_b": wo_b.astype(bf16)}

    in_maps = []
    for cidx in range(NCORES):
        xs = x[cidx * BL:(cidx + 1) * BL]                    # [BL, T, 68]
        xT = np.ascontiguousarray(xs.transpose(2, 1, 0))     # [68, T, BL]
        x_aug = np.concatenate(
            [xT, np.ones((1, T, BL), np.float32)], axis=0)   # [69, T, BL]
        x_aug = x_aug.reshape(KX, T * BL)
        xm = _pad_k(x_aug).astype(bf16)                      # [128, T*BL]
        in_maps.append({"xin": xm, **shared})

    res = bass_utils.run_bass_kernel_spmd(
        nc, in_maps, core_ids=list(range(NCORES)), trace=_want_trace)
    outs = []
    for cidx in range(NCORES):
        o = res.results[cidx]["out"].astype(np.float64)    # [3, 2, 2, BL]
        pool = o.sum(axis=(1, 2))                             # [3, BL]
        out_core = pool.T + fc_out_b                          # [BL, 3]
        outs.append(out_core)
    full = np.concatenate(outs, axis=0).astype(np.float32)
    if _want_trace:
        _CACHE["last_result"] = res
    return full


# revision 14
# speedup vs baseline: 1.0200x; 1.0016x over previous
"""Trainium2 Bass kernel for bidirectional ActionLSTM.

Full inputs in, full output out. Internally: data-parallel over batch
(8 NeuronCores x 256 batch rows), LSTM weights replicated.

Device program (per core, transposed layout: hidden on partitions,
batch on the free dim):
  - fc_in is folded into the LSTM input weights on the host:
        W_x = w_ih @ fc_in_w  [512, 68],  bias = w_ih@fc_in_b + b_ih + b_hh
    and the bias is folded in as an extra all-ones input row (K=69).
  - Gate order permuted to [i, f, o, g]. tanh(g) is computed via the
    identity tanh(z) = 2*sigmoid(2z) - 1, with the 2z folded into the
    g-gate weight rows, so ONE sigmoid activation op covers all gates.
  - tanh(c) on ACT as a separate 256-col instruction (provably minimal:
    tanh(c(t)) must land strictly between SIG(t) and SIG(t+1)).
  - Mean-pool over time + fc_out done ON DEVICE: per step, two tiny
    PE matmuls project h through fc_out^T (M=3) and accumulate in one
    PSUM bank across all 128 steps.  This frees GpSimd completely
    (v2 ran hsum there, which stalled DVE h-writes ~1us/step via the
    shared SBUF port + lazy WAR semaphores).

v3 restructure vs v2:
  - PSUM: one shared 3-tile ring for both dirs' gates (6 banks) + 1
    bank for the fc_out projection accumulator.  Ring slot (d,t) is
    recycled 1.5 steps after its SIG consumes it.
  - GpSimd completely idle (no hsum); SBUF pools deepened (bufs=6) so
    WAR waits never land on the critical chain.
  - Emission in true temporal phase order per step:
      ACT:  SIG_f(t), TANH_b(t-1), TANH_f(t), SIG_b(t)
      DVE:  tt_f, m1h_f, h_b(t-1), add_f, h_f, tt_b, m1h_b, add_b
      PE :  U_b(t), proj_b(t-1), x_b(t+1), U_f(t+1), proj_f(t), x_f(t+2)
    so no in-order engine FIFO head-blocks the other direction's chain.
"""

import os
import numpy as np
from contextlib import ExitStack

INPUT, HID, NCLS = 68, 128, 3
B, T = 2048, 128
NCORES = 8
BL = B // NCORES          # 256 batch rows per core
KX = INPUT + 1            # 69 (ones row folds bias in)
KXP = 128                 # x-side K padded to 128: every matmul
                          # has the same [128,128] weight shape, so
                          # the PE never pays a shape-switch refill
G4 = 4 * HID              # 512

CELL_F32 = os.environ.get("LSTM_CELL_F32", "0") == "1"

_CACHE = {}


def _build_program():
    import concourse.bass as bass
    import concourse.tile as tile
    from concourse import bacc, mybir

    f32 = mybir.dt.float32
    bf16 = mybir.dt.bfloat16
    AF = mybir.ActivationFunctionType
    OP = mybir.AluOpType

    cell_dt = f32 if CELL_F32 else bf16

    nc = bacc.Bacc("TRN2", target_bir_lowering=False, debug=False,
                   num_devices=NCORES)

    xin = nc.dram_tensor("xin", [KXP, T * BL], bf16,
                         kind="ExternalInput").ap()
    wx = {}
    wu = {}
    wo = {}
    for d in "fb":
        wx[d] = nc.dram_tensor(f"wx_{d}", [KXP, G4], bf16,
                               kind="ExternalInput").ap()
        wu[d] = nc.dram_tensor(f"wu_{d}", [HID, G4], bf16,
                               kind="ExternalInput").ap()
        wo[d] = nc.dram_tensor(f"wo_{d}", [HID, NCLS], bf16,
                               kind="ExternalInput").ap()
    out = nc.dram_tensor("out", [NCLS, 2, 2, BL], f32,
                         kind="ExternalOutput").ap()

    with tile.TileContext(nc) as tc, ExitStack() as ctx:
        const = ctx.enter_context(tc.tile_pool(name="const", bufs=1))
        X = const.tile([KXP, T * BL], bf16, tag="X")

        # weights FIRST: they gate the very first matmuls, and anything
        # queued behind the big X transfer waits ~60us.
        WX = {}
        WU = {}
        WO = {}
        for d in "fb":
            WX[d] = const.tile([KXP, G4], bf16, tag=f"wx{d}", name=f"WX{d}")
            nc.sync.dma_start(WX[d][:], wx[d][:])

        # split the big input DMA into chunks so it spreads across DMA
        # queues and so early timesteps unblock compute quickly; issue
        # from both ends since the bwd direction consumes t=T-1 first.
        NCHUNK = 64
        CW = T * BL // NCHUNK
        order = []
        for i in range(NCHUNK // 2):
            order += [i, NCHUNK - 1 - i]
        for ci in order[:2]:
            nc.sync.dma_start(X[:, ci * CW:(ci + 1) * CW],
                              xin[:, ci * CW:(ci + 1) * CW])
        for d in "fb":
            WU[d] = const.tile([HID, G4], bf16, tag=f"wu{d}", name=f"WU{d}")
            nc.sync.dma_start(WU[d][:], wu[d][:])
        for ci in order[2:8]:
            nc.sync.dma_start(X[:, ci * CW:(ci + 1) * CW],
                              xin[:, ci * CW:(ci + 1) * CW])
        for d in "fb":
            WO[d] = const.tile([HID, NCLS], bf16, tag=f"wo{d}", name=f"WO{d}")
            nc.sync.dma_start(WO[d][:], wo[d][:])
        for ci in order[8:]:
            nc.sync.dma_start(X[:, ci * CW:(ci + 1) * CW],
                              xin[:, ci * CW:(ci + 1) * CW])

        hpool = ctx.enter_context(tc.tile_pool(name="h", bufs=6))
        cpool = ctx.enter_context(tc.tile_pool(name="c", bufs=6))
        spool = ctx.enter_context(tc.tile_pool(name="s", bufs=4))
        scpool = ctx.enter_context(tc.tile_pool(name="sc", bufs=6))
        mpool = ctx.enter_context(tc.tile_pool(name="m1h", bufs=6))
        tpool = ctx.enter_context(tc.tile_pool(name="tt", bufs=6))
        gpsum = ctx.enter_context(tc.tile_pool(name="gates", bufs=3,
                                               space="PSUM"))
        ppsum = ctx.enter_context(tc.tile_pool(name="proj", bufs=1,
                                               space="PSUM"))

        proj = ppsum.tile([NCLS, 2, 2, BL], f32, tag="proj", name="proj")
        DI = {"f": 0, "b": 1}

        h = {}
        c = {}
        warm = scpool.tile([HID, 4], cell_dt, tag="warm", name="warm")
        nc.vector.memset(warm[:], 0.0)
        nc.scalar.activation(warm[:], warm[:], AF.Sigmoid)
        jw = const.tile([KXP, BL], bf16, tag="jw", name="jw")
        nc.vector.memset(jw[:], 0.0)
        jg = gpsum.tile([HID, 4, BL], f32, tag="g", name="warmg")
        for i in range(16):
            nc.tensor.matmul(jg[:, i % 4, :], jw[:, 0:HID], jw[:],
                             start=True, stop=(i == 15),
                             skip_group_check=True)
        for d in "fb":
            c[d] = cpool.tile([HID, BL], cell_dt, tag=f"c{d}", name=f"c0{d}")
            nc.vector.memset(c[d][:], 0.0)

        g_cur = {}

        def emit_x(d, t):
            # x-side matmuls for step t into a fresh ring PSUM tile.
            # Each gate pair shares a 2KB PSUM bank: the even gate's
            # start=True matmul pending-zeroes the whole bank, the odd
            # gate accumulates into its (still pending-zero) half.
            g = gpsum.tile([HID, 4, BL], f32, tag="g", name=f"g_{d}_{t}")
            ts = t if d == "f" else T - 1 - t
            for gi in range(4):
                nc.tensor.matmul(g[:, gi, :],
                                 WX[d][:, gi * HID:(gi + 1) * HID],
                                 X[:, ts * BL:(ts + 1) * BL],
                                 start=(gi % 2 == 0),
                                 stop=(t == 0 and gi % 2 == 1),
                                 skip_group_check=True)
            return g

        def emit_U(d, t):
            g = g_cur[d]
            for gi in range(4):
                nc.tensor.matmul(g[:, gi, :],
                                 WU[d][:, gi * HID:(gi + 1) * HID],
                                 h[d], start=False,
                                 stop=(gi % 2 == 1),
                                 skip_group_check=True)

        def emit_sig(d, t, g):
            s = spool.tile([HID, 4, BL], cell_dt, tag=f"s{d}",
                           name=f"s{d}{t}")
            nc.scalar.activation(s[:], g[:], AF.Sigmoid)
            return s

        def emit_cell(d, t, s):
            # c_new = sig(f)*c + (sig(2g)-0.5)*sig(i)   [tt + m1h]
            tt = tpool.tile([HID, BL], cell_dt, tag=f"tt{d}",
                            name=f"tt{d}{t}")
            nc.vector.tensor_tensor(tt[:], s[:, 1, :], c[d][:], op=OP.mult)
            m1h = mpool.tile([HID, BL], cell_dt, tag=f"m{d}",
                             name=f"m{d}{t}")
            nc.vector.scalar_tensor_tensor(m1h[:], s[:, 3, :], 0.5,
                                           s[:, 0, :],
                                           op0=OP.subtract, op1=OP.mult)
            c_new = cpool.tile([HID, BL], cell_dt, tag=f"c{d}",
                               name=f"c{d}{t}")
            nc.vector.tensor_tensor(c_new[:], m1h[:], tt[:], op=OP.add)
            c[d] = c_new
            return c_new

        def emit_tanh(d, t, c_new):
            # sc = tanh(2*c_half) = tanh(c_true); shares the sigmoid
            # ACT table (no reload).
            sc = scpool.tile([HID, BL], cell_dt, tag=f"sc{d}",
                             name=f"sc{d}{t}")
            nc.scalar.activation(sc[:], c_new[:], AF.Tanh, scale=2.0)
            return sc

        ht = {}

        def emit_h(d, t, sc, s):
            if t % 2 == 0:
                ht[d] = hpool.tile([HID, 2, BL], bf16, tag=f"h{d}",
                                   name=f"h{d}{t}")
            nc.vector.tensor_tensor(ht[d][:, t % 2, :], sc[:], s[:, 2, :],
                                    op=OP.mult)
            h[d] = ht[d][:, t % 2, :]

        pending_proj = []

        def emit_proj(d, t):
            # defer emission ~2 iterations so the scheduler ranks these
            # after the chain-critical U matmuls of the next steps
            if t % 2 == 0:
                return
            pending_proj.append((d, t, ht[d]))

        def flush_proj(upto_t):
            while pending_proj and pending_proj[0][1] <= upto_t:
                d, t, htile = pending_proj.pop(0)
                nc.tensor.matmul(proj[:, DI[d], :, :], WO[d][:], htile[:],
                                 start=(t == 1), stop=(t == T - 1),
                                 skip_group_check=True)

        # ---- software-pipelined main loop ----
        # state carried between iterations (dir b runs half a phase
        # behind dir f in the steady schedule):
        g_cur["f"] = emit_x("f", 0)
        g_cur["b"] = emit_x("b", 0)

        s_b = None       # s tile of dir b from step t-1
        sc_b = None      # sc tile of dir b from step t-1
        for t in range(T):
            # --- dir f, step t ---
            if t > 0:
                emit_U("f", t)
            g_f = g_cur["f"]
            if t + 1 < T:
                g_cur["f"] = emit_x("f", t + 1)
            s_f = emit_sig("f", t, g_f)
            c_f = emit_cell("f", t, s_f)
            # dir b finishing step t-1: h_b, U_b of step t
            if t > 0:
                emit_h("b", t - 1, sc_b, s_b)
                emit_U("b", t)
                emit_proj("b", t - 1)
            sc_f = emit_tanh("f", t, c_f)
            # --- dir b, step t ---
            g_b = g_cur["b"]
            if t + 1 < T:
                g_cur["b"] = emit_x("b", t + 1)
            s_b = emit_sig("b", t, g_b)
            emit_h("f", t, sc_f, s_f)
            c_b = emit_cell("b", t, s_b)
            sc_b = emit_tanh("b", t, c_b)
            emit_proj("f", t)
            flush_proj(t - 4 if t < T - 3 else t - 1)
        # drain dir b's final step
        emit_h("b", T - 1, sc_b, s_b)
        emit_proj("b", T - 1)
        flush_proj(T)

        pout = const.tile([NCLS, 2, 2, BL], f32, tag="pout", name="pout")
        nc.vector.tensor_copy(pout[:], proj[:])
        nc.sync.dma_start(out[:], pout[:])

    nc.compile()
    return nc


def _prep_weights(w_ih, w_hh, b_ih, b_hh, fc_in_w, fc_in_b):
    Wx = w_ih.astype(np.float64) @ fc_in_w.astype(np.float64)   # [512, 68]
    bias = w_ih.astype(np.float64) @ fc_in_b.astype(np.float64) \
        + b_ih.astype(np.float64) + b_hh.astype(np.float64)
    perm = np.concatenate([np.arange(0, 128), np.arange(128, 256),
                           np.arange(384, 512), np.arange(256, 384)])
    Wx = Wx[perm]
    U = w_hh.astype(np.float64)[perm]
    bias = bias[perm]
    srow = np.ones((512, 1), np.float64)
    srow[384:] = 2.0
    Wx_aug = np.concatenate([Wx, bias[:, None]], axis=1)        # [512, 69]
    lhsT_x = np.ascontiguousarray((srow * Wx_aug).T)            # [69, 512]
    # h is stored as h = o*tanh(c_true): U scale = 1
    lhsT_U = np.ascontiguousarray((srow * U).T)                 # [128, 512]
    return lhsT_x, lhsT_U


def _pad_k(a):
    # [69, N] -> [128, N] zero-padded (the matching weight rows are zero)
    out = np.zeros((KXP, a.shape[1]), a.dtype)
    out[:a.shape[0]] = a
    return out


def kernel(x, fc_in_w, fc_in_b, w_ih_f, w_hh_f, b_ih_f, b_hh_f,
           w_ih_b, w_hh_b, b_ih_b, b_hh_b, fc_out_w, fc_out_b,
           _want_trace=False):
    from concourse import bass_utils
    import ml_dtypes

    bf16 = ml_dtypes.bfloat16

    if "nc" not in _CACHE:
        _CACHE["nc"] = _build_program()
    nc = _CACHE["nc"]

    lx_f, lU_f = _prep_weights(w_ih_f, w_hh_f, b_ih_f, b_hh_f,
                               fc_in_w, fc_in_b)
    lx_b, lU_b = _prep_weights(w_ih_b, w_hh_b, b_ih_b, b_hh_b,
                               fc_in_w, fc_in_b)
    # on-device pooling + fc_out: wo tiles are fc_out^T * (1/T)
    wo_f = np.ascontiguousarray(fc_out_w[:, :HID].astype(np.float64).T / T)
    wo_b = np.ascontiguousarray(fc_out_w[:, HID:].astype(np.float64).T / T)
    shared = {"wx_f": _pad_k(lx_f).astype(bf16),
              "wx_b": _pad_k(lx_b).astype(bf16),
              "wu_f": lU_f.astype(bf16),
              "wu_b": lU_b.astype(bf16),
              "wo_f": wo_f.astype(bf16),
              "wo_b": wo_b.astype(bf16)}

    in_maps = []
    for cidx in range(NCORES):
        xs = x[cidx * BL:(cidx + 1) * BL]                    # [BL, T, 68]
        xT = np.ascontiguousarray(xs.transpose(2, 1, 0))     # [68, T, BL]
        x_aug = np.concatenate(
            [xT, np.ones((1, T, BL), np.float32)], axis=0)   # [69, T, BL]
        x_aug = x_aug.reshape(KX, T * BL)
        xm = _pad_k(x_aug).astype(bf16)                      # [128, T*BL]
        in_maps.append({"xin": xm, **shared})

    res = bass_utils.run_bass_kernel_spmd(
        nc, in_maps, core_ids=list(range(NCORES)), trace=_want_trace)
    outs = []
    for cidx in range(NCORES):
        o = res.results[cidx]["out"].astype(np.float64)    # [3, 2, 2, BL]
        pool = o.sum(axis=(1, 2))                             # [3, BL]
        out_core = pool.T + fc_out_b                          # [BL, 3]
        outs.append(out_core)
    full = np.concatenate(outs, axis=0).astype(np.float32)
    if _want_trace:
        _CACHE["last_result"] = res
    return full


# revision 16
# speedup vs baseline: 1.0213x; 1.0013x over previous
"""Trainium2 Bass kernel for bidirectional ActionLSTM.

Full inputs in, full output out. Internally: data-parallel over batch
(8 NeuronCores x 256 batch rows), LSTM weights replicated.

Device program (per core, transposed layout: hidden on partitions,
batch on the free dim):
  - fc_in is folded into the LSTM input weights on the host:
        W_x = w_ih @ fc_in_w  [512, 68],  bias = w_ih@fc_in_b + b_ih + b_hh
    and the bias is folded in as an extra all-ones input row (K=69).
  - Gate order permuted to [i, f, o, g]. tanh(g) is computed via the
    identity tanh(z) = 2*sigmoid(2z) - 1, with the 2z folded into the
    g-gate weight rows, so ONE sigmoid activation op covers all gates.
  - tanh(c) on ACT as a separate 256-col instruction (provably minimal:
    tanh(c(t)) must land strictly between SIG(t) and SIG(t+1)).
  - Mean-pool over time + fc_out done ON DEVICE: per step, two tiny
    PE matmuls project h through fc_out^T (M=3) and accumulate in one
    PSUM bank across all 128 steps.  This frees GpSimd completely
    (v2 ran hsum there, which stalled DVE h-writes ~1us/step via the
    shared SBUF port + lazy WAR semaphores).

v3 restructure vs v2:
  - PSUM: one shared 3-tile ring for both dirs' gates (6 banks) + 1
    bank for the fc_out projection accumulator.  Ring slot (d,t) is
    recycled 1.5 steps after its SIG consumes it.
  - GpSimd completely idle (no hsum); SBUF pools deepened (bufs=6) so
    WAR waits never land on the critical chain.
  - Emission in true temporal phase order per step:
      ACT:  SIG_f(t), TANH_b(t-1), TANH_f(t), SIG_b(t)
      DVE:  tt_f, m1h_f, h_b(t-1), add_f, h_f, tt_b, m1h_b, add_b
      PE :  U_b(t), proj_b(t-1), x_b(t+1), U_f(t+1), proj_f(t), x_f(t+2)
    so no in-order engine FIFO head-blocks the other direction's chain.
"""

import os
import numpy as np
from contextlib import ExitStack

INPUT, HID, NCLS = 68, 128, 3
B, T = 2048, 128
NCORES = 8
BL = B // NCORES          # 256 batch rows per core
KX = INPUT + 1            # 69 (ones row folds bias in)
KXP = 128                 # x-side K padded to 128: every matmul
                          # has the same [128,128] weight shape, so
                          # the PE never pays a shape-switch refill
G4 = 4 * HID              # 512

CELL_F32 = os.environ.get("LSTM_CELL_F32", "0") == "1"

_CACHE = {}


def _build_program():
    import concourse.bass as bass
    import concourse.tile as tile
    from concourse import bacc, mybir

    f32 = mybir.dt.float32
    bf16 = mybir.dt.bfloat16
    AF = mybir.ActivationFunctionType
    OP = mybir.AluOpType

    cell_dt = f32 if CELL_F32 else bf16

    nc = bacc.Bacc("TRN2", target_bir_lowering=False, debug=False,
                   num_devices=NCORES)

    xin = nc.dram_tensor("xin", [KXP, T * BL], bf16,
                         kind="ExternalInput").ap()
    wx = {}
    wu = {}
    wo = {}
    for d in "fb":
        wx[d] = nc.dram_tensor(f"wx_{d}", [KXP, G4], bf16,
                               kind="ExternalInput").ap()
        wu[d] = nc.dram_tensor(f"wu_{d}", [HID, G4], bf16,
                               kind="ExternalInput").ap()
        wo[d] = nc.dram_tensor(f"wo_{d}", [HID, NCLS], bf16,
                               kind="ExternalInput").ap()
    out = nc.dram_tensor("out", [NCLS, 2, 2, BL], f32,
                         kind="ExternalOutput").ap()

    with tile.TileContext(nc) as tc, ExitStack() as ctx:
        const = ctx.enter_context(tc.tile_pool(name="const", bufs=1))
        X = const.tile([KXP, T * BL], bf16, tag="X")

        # weights FIRST: they gate the very first matmuls, and anything
        # queued behind the big X transfer waits ~60us.
        WX = {}
        WU = {}
        WO = {}
        for d in "fb":
            WX[d] = const.tile([KXP, G4], bf16, tag=f"wx{d}", name=f"WX{d}")
            nc.sync.dma_start(WX[d][:], wx[d][:])

        # split the big input DMA into chunks so it spreads across DMA
        # queues and so early timesteps unblock compute quickly; issue
        # from both ends since the bwd direction consumes t=T-1 first.
        NCHUNK = 64
        CW = T * BL // NCHUNK
        order = []
        for i in range(NCHUNK // 2):
            order += [i, NCHUNK - 1 - i]
        for st in (0, T - 1, 1, T - 2):
            nc.sync.dma_start(X[:, st * BL:(st + 1) * BL],
                              xin[:, st * BL:(st + 1) * BL])
        for d in "fb":
            WU[d] = const.tile([HID, G4], bf16, tag=f"wu{d}", name=f"WU{d}")
            nc.sync.dma_start(WU[d][:], wu[d][:])
        for ci in order[:8]:
            if ci in (0, NCHUNK - 1):
                continue
            nc.sync.dma_start(X[:, ci * CW:(ci + 1) * CW],
                              xin[:, ci * CW:(ci + 1) * CW])
        for d in "fb":
            WO[d] = const.tile([HID, NCLS], bf16, tag=f"wo{d}", name=f"WO{d}")
            nc.sync.dma_start(WO[d][:], wo[d][:])
        for ci in order[8:]:
            nc.sync.dma_start(X[:, ci * CW:(ci + 1) * CW],
                              xin[:, ci * CW:(ci + 1) * CW])

        hpool = ctx.enter_context(tc.tile_pool(name="h", bufs=6))
        cpool = ctx.enter_context(tc.tile_pool(name="c", bufs=6))
        spool = ctx.enter_context(tc.tile_pool(name="s", bufs=4))
        scpool = ctx.enter_context(tc.tile_pool(name="sc", bufs=6))
        mpool = ctx.enter_context(tc.tile_pool(name="m1h", bufs=6))
        tpool = ctx.enter_context(tc.tile_pool(name="tt", bufs=6))
        gpsum = ctx.enter_context(tc.tile_pool(name="gates", bufs=3,
                                               space="PSUM"))
        ppsum = ctx.enter_context(tc.tile_pool(name="proj", bufs=1,
                                               space="PSUM"))

        proj = ppsum.tile([NCLS, 2, 2, BL], f32, tag="proj", name="proj")
        DI = {"f": 0, "b": 1}

        h = {}
        c = {}
        warm = scpool.tile([HID, 4], cell_dt, tag="warm", name="warm")
        nc.vector.memset(warm[:], 0.0)
        nc.scalar.activation(warm[:], warm[:], AF.Sigmoid)
        jw = const.tile([KXP, BL], bf16, tag="jw", name="jw")
        nc.vector.memset(jw[:], 0.0)
        jg = gpsum.tile([HID, 4, BL], f32, tag="g", name="warmg")
        for i in range(16):
            nc.tensor.matmul(jg[:, i % 4, :], jw[:, 0:HID], jw[:],
                             start=True, stop=(i == 15),
                             skip_group_check=True)
        for d in "fb":
            c[d] = cpool.tile([HID, BL], cell_dt, tag=f"c{d}", name=f"c0{d}")
            nc.vector.memset(c[d][:], 0.0)

        g_cur = {}

        def emit_x(d, t):
            # x-side matmuls for step t into a fresh ring PSUM tile.
            # Each gate pair shares a 2KB PSUM bank: the even gate's
            # start=True matmul pending-zeroes the whole bank, the odd
            # gate accumulates into its (still pending-zero) half.
            g = gpsum.tile([HID, 4, BL], f32, tag="g", name=f"g_{d}_{t}")
            ts = t if d == "f" else T - 1 - t
            for gi in range(4):
                nc.tensor.matmul(g[:, gi, :],
                                 WX[d][:, gi * HID:(gi + 1) * HID],
                                 X[:, ts * BL:(ts + 1) * BL],
                                 start=(gi % 2 == 0),
                                 stop=(t == 0 and gi % 2 == 1),
                                 skip_group_check=True)
            return g

        def emit_U(d, t):
            g = g_cur[d]
            for gi in range(4):
                nc.tensor.matmul(g[:, gi, :],
                                 WU[d][:, gi * HID:(gi + 1) * HID],
                                 h[d], start=False,
                                 stop=(gi % 2 == 1),
                                 skip_group_check=True)

        def emit_sig(d, t, g):
            s = spool.tile([HID, 4, BL], cell_dt, tag=f"s{d}",
                           name=f"s{d}{t}")
            nc.scalar.activation(s[:], g[:], AF.Sigmoid)
            return s

        def emit_cell(d, t, s):
            # c_new = sig(f)*c + (sig(2g)-0.5)*sig(i)   [tt + m1h]
            tt = tpool.tile([HID, BL], cell_dt, tag=f"tt{d}",
                            name=f"tt{d}{t}")
            nc.vector.tensor_tensor(tt[:], s[:, 1, :], c[d][:], op=OP.mult)
            m1h = mpool.tile([HID, BL], cell_dt, tag=f"m{d}",
                             name=f"m{d}{t}")
            nc.vector.scalar_tensor_tensor(m1h[:], s[:, 3, :], 0.5,
                                           s[:, 0, :],
                                           op0=OP.subtract, op1=OP.mult)
            c_new = cpool.tile([HID, BL], cell_dt, tag=f"c{d}",
                               name=f"c{d}{t}")
            nc.vector.tensor_tensor(c_new[:], m1h[:], tt[:], op=OP.add)
            c[d] = c_new
            return c_new

        def emit_tanh(d, t, c_new):
            # sc = tanh(2*c_half) = tanh(c_true); shares the sigmoid
            # ACT table (no reload).
            sc = scpool.tile([HID, BL], cell_dt, tag=f"sc{d}",
                             name=f"sc{d}{t}")
            nc.scalar.activation(sc[:], c_new[:], AF.Tanh, scale=2.0)
            return sc

        ht = {}

        def emit_h(d, t, sc, s):
            if t % 2 == 0:
                ht[d] = hpool.tile([HID, 2, BL], bf16, tag=f"h{d}",
                                   name=f"h{d}{t}")
            nc.vector.tensor_tensor(ht[d][:, t % 2, :], sc[:], s[:, 2, :],
                                    op=OP.mult)
            h[d] = ht[d][:, t % 2, :]

        pending_proj = []

        def emit_proj(d, t):
            # defer emission ~2 iterations so the scheduler ranks these
            # after the chain-critical U matmuls of the next steps
            if t % 2 == 0:
                return
            pending_proj.append((d, t, ht[d]))

        def flush_proj(upto_t):
            while pending_proj and pending_proj[0][1] <= upto_t:
                d, t, htile = pending_proj.pop(0)
                nc.tensor.matmul(proj[:, DI[d], :, :], WO[d][:], htile[:],
                                 start=(t == 1), stop=(t == T - 1),
                                 skip_group_check=True)

        # ---- software-pipelined main loop ----
        # state carried between iterations (dir b runs half a phase
        # behind dir f in the steady schedule):
        g_cur["f"] = emit_x("f", 0)
        g_cur["b"] = emit_x("b", 0)

        s_b = None       # s tile of dir b from step t-1
        sc_b = None      # sc tile of dir b from step t-1
        for t in range(T):
            # --- dir f, step t ---
            if t > 0:
                emit_U("f", t)
            g_f = g_cur["f"]
            if t + 1 < T:
                g_cur["f"] = emit_x("f", t + 1)
            s_f = emit_sig("f", t, g_f)
            c_f = emit_cell("f", t, s_f)
            # dir b finishing step t-1: h_b, U_b of step t
            if t > 0:
                emit_h("b", t - 1, sc_b, s_b)
                emit_U("b", t)
                emit_proj("b", t - 1)
            sc_f = emit_tanh("f", t, c_f)
            # --- dir b, step t ---
            g_b = g_cur["b"]
            if t + 1 < T:
                g_cur["b"] = emit_x("b", t + 1)
            s_b = emit_sig("b", t, g_b)
            emit_h("f", t, sc_f, s_f)
            c_b = emit_cell("b", t, s_b)
            sc_b = emit_tanh("b", t, c_b)
            emit_proj("f", t)
            flush_proj(t - 4 if t < T - 3 else t - 1)
        # drain dir b's final step
        emit_h("b", T - 1, sc_b, s_b)
        emit_proj("b", T - 1)
        flush_proj(T)

        pout = const.tile([NCLS, 2, 2, BL], f32, tag="pout", name="pout")
        nc.vector.tensor_copy(pout[:], proj[:])
        nc.sync.dma_start(out[:], pout[:])

    nc.compile()
    return nc


def _prep_weights(w_ih, w_hh, b_ih, b_hh, fc_in_w, fc_in_b):
    Wx = w_ih.astype(np.float64) @ fc_in_w.astype(np.float64)   # [512, 68]
    bias = w_ih.astype(np.float64) @ fc_in_b.astype(np.float64) \
        + b_ih.astype(np.float64) + b_hh.astype(np.float64)
    perm = np.concatenate([np.arange(0, 128), np.arange(128, 256),
                           np.arange(384, 512), np.arange(256, 384)])
    Wx = Wx[perm]
    U = w_hh.astype(np.float64)[perm]
    bias = bias[perm]
    srow = np.ones((512, 1), np.float64)
    srow[384:] = 2.0
    Wx_aug = np.concatenate([Wx, bias[:, None]], axis=1)        # [512, 69]
    lhsT_x = np.ascontiguousarray((srow * Wx_aug).T)            # [69, 512]
    # h is stored as h = o*tanh(c_true): U scale = 1
    lhsT_U = np.ascontiguousarray((srow * U).T)                 # [128, 512]
    return lhsT_x, lhsT_U


def _pad_k(a):
    # [69, N] -> [128, N] zero-padded (the matching weight rows are zero)
    out = np.zeros((KXP, a.shape[1]), a.dtype)
    out[:a.shape[0]] = a
    return out


def kernel(x, fc_in_w, fc_in_b, w_ih_f, w_hh_f, b_ih_f, b_hh_f,
           w_ih_b, w_hh_b, b_ih_b, b_hh_b, fc_out_w, fc_out_b,
           _want_trace=False):
    from concourse import bass_utils
    import ml_dtypes

    bf16 = ml_dtypes.bfloat16

    if "nc" not in _CACHE:
        _CACHE["nc"] = _build_program()
    nc = _CACHE["nc"]

    lx_f, lU_f = _prep_weights(w_ih_f, w_hh_f, b_ih_f, b_hh_f,
                               fc_in_w, fc_in_b)
    lx_b, lU_b = _prep_weights(w_ih_b, w_hh_b, b_ih_b, b_hh_b,
                               fc_in_w, fc_in_b)
    # on-device pooling + fc_out: wo tiles are fc_out^T * (1/T)
    wo_f = np.ascontiguousarray(fc_out_w[:, :HID].astype(np.float64).T / T)
    wo_b = np.ascontiguousarray(fc_out_w[:, HID:].astype(np.float64).T / T)
    shared = {"wx_f": _pad_k(lx_f).astype(bf16),
              "wx_b": _pad_k(lx_b).astype(bf16),
              "wu_f": lU_f.astype(bf16),
              "wu_b": lU_b.astype(bf16),
              "wo_f": wo_f.astype(bf16),
              "wo_b": wo_b.astype(bf16)}

    in_maps = []
    for cidx in range(NCORES):
        xs = x[cidx * BL:(cidx + 1) * BL]                    # [BL, T, 68]
        xT = np.ascontiguousarray(xs.transpose(2, 1, 0))     # [68, T, BL]
        x_aug = np.concatenate(
            [xT, np.ones((1, T, BL), np.float32)], axis=0)   # [69, T, BL]
        x_aug = x_aug.reshape(KX, T * BL)
        xm = _pad_k(x_aug).astype(bf16)                      # [128, T*BL]
        in_maps.append({"xin": xm, **shared})

    res = bass_utils.run_bass_kernel_spmd(
        nc, in_maps, core_ids=list(range(NCORES)), trace=_want_trace)
    outs = []
    for cidx in range(NCORES):
        o = res.results[cidx]["out"].astype(np.float64)    # [3, 2, 2, BL]
        pool = o.sum(axis=(1, 2))                             # [3, BL]
        out_core = pool.T + fc_out_b                          # [BL, 3]
        outs.append(out_core)
    full = np.concatenate(outs, axis=0).astype(np.float32)
    if _want_trace:
        _CACHE["last_result"] = res
    return full


# revision 17
# speedup vs baseline: 1.0214x; 1.0000x over previous
"""Trainium2 Bass kernel for bidirectional ActionLSTM.

Full inputs in, full output out. Internally: data-parallel over batch
(8 NeuronCores x 256 batch rows), LSTM weights replicated.

Device program (per core, transposed layout: hidden on partitions,
batch on the free dim):
  - fc_in is folded into the LSTM input weights on the host:
        W_x = w_ih @ fc_in_w  [512, 68],  bias = w_ih@fc_in_b + b_ih + b_hh
    and the bias is folded in as an extra all-ones input row (K=69).
  - Gate order permuted to [i, f, o, g]. tanh(g) is computed via the
    identity tanh(z) = 2*sigmoid(2z) - 1, with the 2z folded into the
    g-gate weight rows, so ONE sigmoid activation op covers all gates.
  - tanh(c) on ACT as a separate 256-col instruction (provably minimal:
    tanh(c(t)) must land strictly between SIG(t) and SIG(t+1)).
  - Mean-pool over time + fc_out done ON DEVICE: per step, two tiny
    PE matmuls project h through fc_out^T (M=3) and accumulate in one
    PSUM bank across all 128 steps.  This frees GpSimd completely
    (v2 ran hsum there, which stalled DVE h-writes ~1us/step via the
    shared SBUF port + lazy WAR semaphores).

v3 restructure vs v2:
  - PSUM: one shared 3-tile ring for both dirs' gates (6 banks) + 1
    bank for the fc_out projection accumulator.  Ring slot (d,t) is
    recycled 1.5 steps after its SIG consumes it.
  - GpSimd completely idle (no hsum); SBUF pools deepened (bufs=6) so
    WAR waits never land on the critical chain.
  - Emission in true temporal phase order per step:
      ACT:  SIG_f(t), TANH_b(t-1), TANH_f(t), SIG_b(t)
      DVE:  tt_f, m1h_f, h_b(t-1), add_f, h_f, tt_b, m1h_b, add_b
      PE :  U_b(t), proj_b(t-1), x_b(t+1), U_f(t+1), proj_f(t), x_f(t+2)
    so no in-order engine FIFO head-blocks the other direction's chain.
"""

import os
import numpy as np
from contextlib import ExitStack

INPUT, HID, NCLS = 68, 128, 3
B, T = 2048, 128
NCORES = 8
BL = B // NCORES          # 256 batch rows per core
KX = INPUT + 1            # 69 (ones row folds bias in)
KXP = 128                 # x-side K padded to 128: every matmul
                          # has the same [128,128] weight shape, so
                          # the PE never pays a shape-switch refill
G4 = 4 * HID              # 512

CELL_F32 = os.environ.get("LSTM_CELL_F32", "0") == "1"

_CACHE = {}


def _build_program():
    import concourse.bass as bass
    import concourse.tile as tile
    from concourse import bacc, mybir

    f32 = mybir.dt.float32
    bf16 = mybir.dt.bfloat16
    AF = mybir.ActivationFunctionType
    OP = mybir.AluOpType

    cell_dt = f32 if CELL_F32 else bf16

    nc = bacc.Bacc("TRN2", target_bir_lowering=False, debug=False,
                   num_devices=NCORES)

    xin = nc.dram_tensor("xin", [KXP, T * BL], bf16,
                         kind="ExternalInput").ap()
    wx = {}
    wu = {}
    wo = {}
    for d in "fb":
        wx[d] = nc.dram_tensor(f"wx_{d}", [KXP, G4], bf16,
                               kind="ExternalInput").ap()
        wu[d] = nc.dram_tensor(f"wu_{d}", [HID, G4], bf16,
                               kind="ExternalInput").ap()
        wo[d] = nc.dram_tensor(f"wo_{d}", [HID, NCLS], bf16,
                               kind="ExternalInput").ap()
    out = {}
    for d in "fb":
        out[d] = nc.dram_tensor(f"out_{d}", [NCLS, 2, BL], f32,
                                kind="ExternalOutput").ap()

    with tile.TileContext(nc) as tc, ExitStack() as ctx:
        const = ctx.enter_context(tc.tile_pool(name="const", bufs=1))
        X = const.tile([KXP, T * BL], bf16, tag="X")

        # weights FIRST: they gate the very first matmuls, and anything
        # queued behind the big X transfer waits ~60us.
        WX = {}
        WU = {}
        WO = {}
        for d in "fb":
            WX[d] = const.tile([KXP, G4], bf16, tag=f"wx{d}", name=f"WX{d}")
            nc.sync.dma_start(WX[d][:], wx[d][:])

        # split the big input DMA into chunks so it spreads across DMA
        # queues and so early timesteps unblock compute quickly; issue
        # from both ends since the bwd direction consumes t=T-1 first.
        NCHUNK = 64
        CW = T * BL // NCHUNK
        order = []
        for i in range(NCHUNK // 2):
            order += [i, NCHUNK - 1 - i]
        for st in (0, T - 1, 1, T - 2):
            nc.sync.dma_start(X[:, st * BL:(st + 1) * BL],
                              xin[:, st * BL:(st + 1) * BL])
        for d in "fb":
            WU[d] = const.tile([HID, G4], bf16, tag=f"wu{d}", name=f"WU{d}")
            nc.sync.dma_start(WU[d][:], wu[d][:])
        for ci in order[:8]:
            if ci in (0, NCHUNK - 1):
                continue
            nc.sync.dma_start(X[:, ci * CW:(ci + 1) * CW],
                              xin[:, ci * CW:(ci + 1) * CW])
        for d in "fb":
            WO[d] = const.tile([HID, NCLS], bf16, tag=f"wo{d}", name=f"WO{d}")
            nc.sync.dma_start(WO[d][:], wo[d][:])
        for ci in order[8:]:
            nc.sync.dma_start(X[:, ci * CW:(ci + 1) * CW],
                              xin[:, ci * CW:(ci + 1) * CW])

        hpool = ctx.enter_context(tc.tile_pool(name="h", bufs=6))
        cpool = ctx.enter_context(tc.tile_pool(name="c", bufs=6))
        spool = ctx.enter_context(tc.tile_pool(name="s", bufs=4))
        scpool = ctx.enter_context(tc.tile_pool(name="sc", bufs=6))
        mpool = ctx.enter_context(tc.tile_pool(name="m1h", bufs=6))
        tpool = ctx.enter_context(tc.tile_pool(name="tt", bufs=6))
        gpsum = ctx.enter_context(tc.tile_pool(name="gates", bufs=3,
                                               space="PSUM"))
        ppsum = ctx.enter_context(tc.tile_pool(name="proj", bufs=1,
                                               space="PSUM"))

        proj = ppsum.tile([NCLS, 2, 2, BL], f32, tag="proj", name="proj")
        DI = {"f": 0, "b": 1}

        h = {}
        c = {}
        warm = scpool.tile([HID, 4], cell_dt, tag="warm", name="warm")
        nc.vector.memset(warm[:], 0.0)
        nc.scalar.activation(warm[:], warm[:], AF.Sigmoid)
        jw = const.tile([KXP, BL], bf16, tag="jw", name="jw")
        nc.vector.memset(jw[:], 0.0)
        jg = gpsum.tile([HID, 4, BL], f32, tag="g", name="warmg")
        for i in range(16):
            nc.tensor.matmul(jg[:, i % 4, :], jw[:, 0:HID], jw[:],
                             start=True, stop=(i == 15),
                             skip_group_check=True)
        for d in "fb":
            c[d] = cpool.tile([HID, BL], cell_dt, tag=f"c{d}", name=f"c0{d}")
            nc.vector.memset(c[d][:], 0.0)

        g_cur = {}

        def emit_x(d, t):
            # x-side matmuls for step t into a fresh ring PSUM tile.
            # Each gate pair shares a 2KB PSUM bank: the even gate's
            # start=True matmul pending-zeroes the whole bank, the odd
            # gate accumulates into its (still pending-zero) half.
            g = gpsum.tile([HID, 4, BL], f32, tag="g", name=f"g_{d}_{t}")
            ts = t if d == "f" else T - 1 - t
            for gi in range(4):
                nc.tensor.matmul(g[:, gi, :],
                                 WX[d][:, gi * HID:(gi + 1) * HID],
                                 X[:, ts * BL:(ts + 1) * BL],
                                 start=(gi % 2 == 0),
                                 stop=(t == 0 and gi % 2 == 1),
                                 skip_group_check=True)
            return g

        def emit_U(d, t):
            g = g_cur[d]
            for gi in range(4):
                nc.tensor.matmul(g[:, gi, :],
                                 WU[d][:, gi * HID:(gi + 1) * HID],
                                 h[d], start=False,
                                 stop=(gi % 2 == 1),
                                 skip_group_check=True)

        def emit_sig(d, t, g):
            s = spool.tile([HID, 4, BL], cell_dt, tag=f"s{d}",
                           name=f"s{d}{t}")
            nc.scalar.activation(s[:], g[:], AF.Sigmoid)
            return s

        def emit_cell(d, t, s):
            # c_new = sig(f)*c + (sig(2g)-0.5)*sig(i)   [tt + m1h]
            tt = tpool.tile([HID, BL], cell_dt, tag=f"tt{d}",
                            name=f"tt{d}{t}")
            nc.vector.tensor_tensor(tt[:], s[:, 1, :], c[d][:], op=OP.mult)
            m1h = mpool.tile([HID, BL], cell_dt, tag=f"m{d}",
                             name=f"m{d}{t}")
            nc.vector.scalar_tensor_tensor(m1h[:], s[:, 3, :], 0.5,
                                           s[:, 0, :],
                                           op0=OP.subtract, op1=OP.mult)
            c_new = cpool.tile([HID, BL], cell_dt, tag=f"c{d}",
                               name=f"c{d}{t}")
            nc.vector.tensor_tensor(c_new[:], m1h[:], tt[:], op=OP.add)
            c[d] = c_new
            return c_new

        def emit_tanh(d, t, c_new):
            # sc = tanh(2*c_half) = tanh(c_true); shares the sigmoid
            # ACT table (no reload).
            sc = scpool.tile([HID, BL], cell_dt, tag=f"sc{d}",
                             name=f"sc{d}{t}")
            nc.scalar.activation(sc[:], c_new[:], AF.Tanh, scale=2.0)
            return sc

        ht = {}

        def emit_h(d, t, sc, s):
            if t % 2 == 0:
                ht[d] = hpool.tile([HID, 2, BL], bf16, tag=f"h{d}",
                                   name=f"h{d}{t}")
            nc.vector.tensor_tensor(ht[d][:, t % 2, :], sc[:], s[:, 2, :],
                                    op=OP.mult)
            h[d] = ht[d][:, t % 2, :]

        pending_proj = []

        def emit_proj(d, t):
            # defer emission ~2 iterations so the scheduler ranks these
            # after the chain-critical U matmuls of the next steps
            if t % 2 == 0:
                return
            pending_proj.append((d, t, ht[d]))

        def flush_proj(upto_t):
            while pending_proj and pending_proj[0][1] <= upto_t:
                d, t, htile = pending_proj.pop(0)
                nc.tensor.matmul(proj[:, DI[d], :, :], WO[d][:], htile[:],
                                 start=(t == 1), stop=(t == T - 1),
                                 skip_group_check=True)

        # ---- software-pipelined main loop ----
        # state carried between iterations (dir b runs half a phase
        # behind dir f in the steady schedule):
        g_cur["f"] = emit_x("f", 0)
        g_cur["b"] = emit_x("b", 0)

        s_b = None       # s tile of dir b from step t-1
        sc_b = None      # sc tile of dir b from step t-1
        for t in range(T):
            # --- dir f, step t ---
            if t > 0:
                emit_U("f", t)
            g_f = g_cur["f"]
            if t + 1 < T:
                g_cur["f"] = emit_x("f", t + 1)
            s_f = emit_sig("f", t, g_f)
            c_f = emit_cell("f", t, s_f)
            # dir b finishing step t-1: h_b, U_b of step t
            if t > 0:
                emit_h("b", t - 1, sc_b, s_b)
                emit_U("b", t)
                emit_proj("b", t - 1)
            sc_f = emit_tanh("f", t, c_f)
            # --- dir b, step t ---
            g_b = g_cur["b"]
            if t + 1 < T:
                g_cur["b"] = emit_x("b", t + 1)
            s_b = emit_sig("b", t, g_b)
            emit_h("f", t, sc_f, s_f)
            c_b = emit_cell("b", t, s_b)
            sc_b = emit_tanh("b", t, c_b)
            emit_proj("f", t)
            flush_proj(t - 4 if t < T - 3 else t - 1)
        # drain dir b's final step
        emit_h("b", T - 1, sc_b, s_b)
        emit_proj("b", T - 1)
        flush_proj(T)

        for d in "fb":
            po = const.tile([NCLS, 2, BL], f32, tag=f"po{d}", name=f"po{d}")
            nc.vector.tensor_copy(po[:], proj[:, DI[d], :, :])
            nc.sync.dma_start(out[d][:], po[:])

    nc.compile()
    return nc


def _prep_weights(w_ih, w_hh, b_ih, b_hh, fc_in_w, fc_in_b):
    Wx = w_ih.astype(np.float64) @ fc_in_w.astype(np.float64)   # [512, 68]
    bias = w_ih.astype(np.float64) @ fc_in_b.astype(np.float64) \
        + b_ih.astype(np.float64) + b_hh.astype(np.float64)
    perm = np.concatenate([np.arange(0, 128), np.arange(128, 256),
                           np.arange(384, 512), np.arange(256, 384)])
    Wx = Wx[perm]
    U = w_hh.astype(np.float64)[perm]
    bias = bias[perm]
    srow = np.ones((512, 1), np.float64)
    srow[384:] = 2.0
    Wx_aug = np.concatenate([Wx, bias[:, None]], axis=1)        # [512, 69]
    lhsT_x = np.ascontiguousarray((srow * Wx_aug).T)            # [69, 512]
    # h is stored as h = o*tanh(c_true): U scale = 1
    lhsT_U = np.ascontiguousarray((srow * U).T)                 # [128, 512]
    return lhsT_x, lhsT_U


def _pad_k(a):
    # [69, N] -> [128, N] zero-padded (the matching weight rows are zero)
    out = np.zeros((KXP, a.shape[1]), a.dtype)
    out[:a.shape[0]] = a
    return out


def kernel(x, fc_in_w, fc_in_b, w_ih_f, w_hh_f, b_ih_f, b_hh_f,
           w_ih_b, w_hh_b, b_ih_b, b_hh_b, fc_out_w, fc_out_b,
           _want_trace=False):
    from concourse import bass_utils
    import ml_dtypes

    bf16 = ml_dtypes.bfloat16

    if "nc" not in _CACHE:
        _CACHE["nc"] = _build_program()
    nc = _CACHE["nc"]

    lx_f, lU_f = _prep_weights(w_ih_f, w_hh_f, b_ih_f, b_hh_f,
                               fc_in_w, fc_in_b)
    lx_b, lU_b = _prep_weights(w_ih_b, w_hh_b, b_ih_b, b_hh_b,
                               fc_in_w, fc_in_b)
    # on-device pooling + fc_out: wo tiles are fc_out^T * (1/T)
    wo_f = np.ascontiguousarray(fc_out_w[:, :HID].astype(np.float64).T / T)
    wo_b = np.ascontiguousarray(fc_out_w[:, HID:].astype(np.float64).T / T)
    shared = {"wx_f": _pad_k(lx_f).astype(bf16),
              "wx_b": _pad_k(lx_b).astype(bf16),
              "wu_f": lU_f.astype(bf16),
              "wu_b": lU_b.astype(bf16),
              "wo_f": wo_f.astype(bf16),
              "wo_b": wo_b.astype(bf16)}

    in_maps = []
    for cidx in range(NCORES):
        xs = x[cidx * BL:(cidx + 1) * BL]                    # [BL, T, 68]
        xT = np.ascontiguousarray(xs.transpose(2, 1, 0))     # [68, T, BL]
        x_aug = np.concatenate(
            [xT, np.ones((1, T, BL), np.float32)], axis=0)   # [69, T, BL]
        x_aug = x_aug.reshape(KX, T * BL)
        xm = _pad_k(x_aug).astype(bf16)                      # [128, T*BL]
        in_maps.append({"xin": xm, **shared})

    res = bass_utils.run_bass_kernel_spmd(
        nc, in_maps, core_ids=list(range(NCORES)), trace=_want_trace)
    outs = []
    for cidx in range(NCORES):
        of = res.results[cidx]["out_f"].astype(np.float64)   # [3, 2, BL]
        ob = res.results[cidx]["out_b"].astype(np.float64)
        pool = of.sum(axis=1) + ob.sum(axis=1)                # [3, BL]
        out_core = pool.T + fc_out_b                          # [BL, 3]
        outs.append(out_core)
    full = np.concatenate(outs, axis=0).astype(np.float32)
    if _want_trace:
        _CACHE["last_result"] = res
    return full
